# revision 15
# baseline (speedup 1.0000x reference)
"""Additive attention (B=4, C=256, CO=64, H=W=24) on 8 TRN2 NeuronCores.

Sharding: core i handles batch b = i // 2 and Nq-half h = i % 2 (rows
12h..12h+12 of the 24x24 query grid). Each core produces a complete
(256, 288) slice of the output; no collectives are needed.

Algorithm (Fourier-factorized additive attention): the score tensor
  scores[k, q] = sum_c wf_c * tanh(k_c[k] + q_c[q])
is O(Nk*Nq*CO) elementwise work if computed directly (the tanh alone is
~69us/core on the ACT engine). Instead approximate
  tanh(x) ~= a*x + sum_r b_r sin(om_r x)
(free-frequency least-squares fit, weighted by the N(0,2) density of
x = k_c + q_c; R=5 gives weighted-RMS error 9.3e-4) and use
  sin(om(k+q)) = sin(om k)cos(om q) + cos(om k)sin(om q),
which factorizes scores into a rank-(2R*CO + 2) matmul:
  scores = F(k)^T G(q) + a*(Ak[k] + Aq[q]),
with F/G = {sin,cos}(om_r * .) feature maps over the 64 channels. The
O(N^2 C) tanh becomes an O(N^2 * 2R*C) PE matmul plus O(N*C*R)
elementwise sin work - engines: PE ~8us, ACT ~7us, DVE ~6us per core.

Range reduction for sin: a custom DVE op (FRAC_SHIFT_ANT, registered at
runtime) computes f = y - round(y) with y = x*(om/2pi) + phase/2pi via
the fp32 magic-constant rounding trick; ACT then evaluates
sin(2pi * f), arg range exactly [-pi, pi] (the ACT Sin table diverges
beyond ~|3.5| rad). cos rides the same op via phase=0.25.

sigmoid(s) is computed as 0.5 + 0.5*tanh(0.5 s) (Sin and Tanh share
one ACT table -> no table reloads); the 0.5 offset becomes a
0.5*rowsum(value) correction added at the output copy, and the 0.5
factor folds into the transposed-value tiles.

Measured: ~? us exec (neuron-profile), predicted rel err ~5e-4.
"""

import numpy as np

B, C, CO, HW = 4, 256, 64, 24
NK = 576
NQ = 288  # per-core query count (half of 576)
KT_SIZES = [128, 128, 128, 128, 64]

# tanh(x) ~= A_LIN*x + sum_r BB[r]*sin(OM[r]*x); weighted LSQ fit on N(0,2)
# R=4: weighted-RMS 2.4e-3, end-to-end rel err ~1.2e-3 (R=5 alt: 0.18780 /
# [0.589796,1.188114,1.868618,2.723939,3.824876] /
# [0.553768,0.196597,0.080735,0.02663,0.006279] -> 5.3e-4)
A_LIN = 0.18960
OM = [0.595782, 1.259669, 2.109728, 3.210177]
BB = [0.561325, 0.210306, 0.069877, 0.016487]
R = len(OM)
TWO_PI = float(2.0 * np.pi)
MAGIC = 12582912.0  # 3 * 2^22: fp32 round-to-nearest-integer constant

_cache = {}


def _register_frac_op():
    """Register the FRAC_SHIFT_ANT custom DVE op (idempotent):
    out = y - round(y), y = in0*s0 + s1  (all fp32; round via +/-MAGIC).
    """
    import concourse.dve_ops as dve_ops
    from concourse.dve_spec import Spec, Src0, C0, C1, C2, lower
    from concourse.dve_uop import DveOpSpec

    for op in dve_ops.OPS:
        if op.name == "FRAC_SHIFT_ANT":
            return op

    y = Src0 * C0 + C1
    n = (y + C2) - C2
    spec = Spec(
        body=y - n,
        reference=lambda in0, in1, s0, s1, imm2: (
            lambda yy: yy
            - ((yy + np.float32(imm2)).astype(np.float32) - np.float32(imm2))
        )((np.float32(in0) * np.float32(s0) + np.float32(s1)).astype(np.float32)),
    )
    opcode = dve_ops._CUSTOM_DVE_ROW_BASE + len(dve_ops.OPS)
    shas = {}
    for ver in ("v3", "v4"):
        shas[ver] = DveOpSpec(
            name="FRAC_SHIFT_ANT", opcode=opcode, uops=lower(spec, ver=ver),
            rd1_en=False,
        ).sha(ver)
    op = dve_ops.DveOp("FRAC_SHIFT_ANT", spec, subdim=False, uops_sha=shas)
    dve_ops.OPS.append(op)
    dve_ops.CUSTOM_DVE_SPECS[op.name] = op.spec
    dve_ops._SUB_OPCODE_FOR_NAME[op.name] = opcode
    return op


def _build():
    import concourse.bacc as bacc
    import concourse.mybir as mybir
    from concourse.tile import TileContext

    frac_op = _register_frac_op()

    f32 = mybir.dt.float32
    f16 = mybir.dt.float16
    AF = mybir.ActivationFunctionType

    nc = bacc.Bacc("TRN2", target_bir_lowering=False, debug=False, num_devices=8)
    with TileContext(nc) as tc:
        kqin = nc.dram_tensor("kqin", [C, NK + NQ], f16, kind="ExternalInput")
        valtin = nc.dram_tensor("valtin", [NK, C], f16, kind="ExternalInput")
        wkq = nc.dram_tensor("wkq", [C, 256], f16, kind="ExternalInput")
        wrapv = nc.dram_tensor("wrapv", [128, 2 * R], f32, kind="ExternalInput")
        qscale = nc.dram_tensor("qscale", [128, R], f32, kind="ExternalInput")
        awfbc = nc.dram_tensor("awfbc", [CO, NQ], f16, kind="ExternalInput")
        bfv = nc.dram_tensor("bfv", [128, 1], f32, kind="ExternalInput")
        vsum05 = nc.dram_tensor("vsum05", [128, 2], f32, kind="ExternalInput")
        outd = nc.dram_tensor("out", [C, NQ], f32, kind="ExternalOutput")

        with (
            tc.tile_pool(name="consts", bufs=1) as consts,
            tc.tile_pool(name="inp", bufs=1) as inp,
            tc.tile_pool(name="work", bufs=1) as work,
        ):
            kq_sb = [inp.tile([128, NK + NQ], f16, tag=f"kq{t}", name=f"kq{t}") for t in range(2)]
            vt_sb = [
                inp.tile([KT_SIZES[kt], C], f16, tag=f"vt{kt}", name=f"vt{kt}")
                for kt in range(5)
            ]
            wkq_sb = [consts.tile([128, 256], f16, tag=f"wkq{t}", name=f"wkq{t}") for t in range(2)]
            wrapv_sb = consts.tile([128, 2 * R], f32, tag="wrapv")
            qscale_sb = consts.tile([128, R], f32, tag="qscale")
            awfbc_sb = consts.tile([CO, NQ], f16, tag="awfbc")
            bfv_sb = consts.tile([128, 1], f32, tag="bfv")
            vs_sb = consts.tile([128, 2], f32, tag="vs")
            scr = consts.tile([128, 1], f32, tag="scr")

            # DMA issue order matters per queue; spread across SP/ACT/Pool.
            # SP: kq0 + small consts; ACT: weights (+ act-table warmups);
            # Pool: kq1, transposed value, rest.
            nc.sync.dma_start(out=kq_sb[0][:], in_=kqin.ap()[0:128, :])
            nc.scalar.dma_start(out=wkq_sb[0][:], in_=wkq.ap()[0:128, :])
            nc.gpsimd.dma_start(out=kq_sb[1][:], in_=kqin.ap()[128:256, :])
            nc.scalar.dma_start(out=wkq_sb[1][:], in_=wkq.ap()[128:256, :])
            nc.sync.dma_start(out=wrapv_sb[:], in_=wrapv.ap())
            nc.sync.dma_start(out=qscale_sb[:], in_=qscale.ap())
            nc.sync.dma_start(out=awfbc_sb[:], in_=awfbc.ap())
            # warmup: Silu exists only in the silu_and_others table (which
            # also holds Sin and Tanh), so one Silu ACT pins that table for
            # the whole kernel - no mid-kernel ACT_TABLE_LOADs. Runs in the
            # DMA shadow.
            nc.vector.memset(scr[:], 0.0)
            nc.scalar.activation(scr[:], scr[:], AF.Silu)
            for kt in range(5):
                nc.gpsimd.dma_start(
                    out=vt_sb[kt][:],
                    in_=valtin.ap()[kt * 128 : kt * 128 + KT_SIZES[kt], :],
                )
            nc.gpsimd.dma_start(out=bfv_sb[:], in_=bfv.ap())
            nc.gpsimd.dma_start(out=vs_sb[:], in_=vsum05.ap())

            # PE p-state warmup: ~3us of dummy matmuls in the DMA shadow so
            # the real matmuls start at full clock
            warm = consts.tile([128, 512], f16, tag="warm")
            nc.vector.memset(warm[:], 0.0)
            with tc.tile_pool(name="pwarm", bufs=1, space="PSUM") as pwarm:
                pw = pwarm.tile([128, 512], f32, tag="pw")
                for i in range(8):
                    nc.tensor.matmul(
                        out=pw[:], lhsT=warm[:, 0:128], rhs=warm[:],
                        start=(i == 0), stop=(i == 7),
                    )

            dup = work.tile([128, NK + NQ], f16, tag="dup")
            attn_sb = [
                work.tile([KT_SIZES[kt], NQ], f16, tag=f"attn{kt}", name=f"attn{kt}")
                for kt in range(5)
            ]
            osb = [work.tile([128, NQ], f32, tag=f"osb{t}", name=f"osb{t}") for t in range(2)]

            with tc.tile_pool(name="psc", bufs=1, space="PSUM") as psc:
              with (
                tc.tile_pool(name="pkq", bufs=1, space="PSUM") as pkq,
                tc.tile_pool(name="wp", bufs=2) as wp,
                tc.tile_pool(name="fp", bufs=2) as fp,
                tc.tile_pool(name="gp", bufs=2) as gp,
              ):
                # k_/q_ = [W|W]^T @ (key|qry) -> duplicated rows, one psum
                # tile; ct0 matmuls first (kq1 DMA lands later)
                pkq_t = pkq.tile([128, NK + NQ], f32, tag="pkq")
                for c0, c1 in ((0, 512), (512, NK)):
                    for ct in range(2):
                        nc.tensor.matmul(
                            out=pkq_t[:, c0:c1], lhsT=wkq_sb[ct][:, 0:128],
                            rhs=kq_sb[ct][:, c0:c1],
                            start=(ct == 0), stop=(ct == 1),
                        )
                for ct in range(2):
                    nc.tensor.matmul(
                        out=pkq_t[:, NK : NK + NQ], lhsT=wkq_sb[ct][:, 128:256],
                        rhs=kq_sb[ct][:, NK : NK + NQ],
                        start=(ct == 0), stop=(ct == 1),
                    )

                scores = [
                    psc.tile([KT_SIZES[kt], NQ], f32, tag=f"sc{kt}", name=f"sc{kt}")
                    for kt in range(5)
                ]

                # DVE order: wraps r=0 first (critical), then the f16 dup
                # copy (feeds only the linear matmuls), then per r:
                # wraps r+1 ahead of gr(r) so DVE never stalls on ACT(r).
                wr_t, fr_t = [None] * R, [None] * R

                def emit_wraps(r):
                    # r=0 reads the psum directly (dup not cast yet); later
                    # r's read the f16 SBUF copy (cheaper DVE access)
                    srcт = pkq_t if r == 0 else dup
                    wr = wp.tile([128, NK + NQ], f32, tag="wr")
                    s0 = float(OM[r] / TWO_PI)
                    nc.vector._custom_dve(
                        frac_op, out=wr[:, 0:NK], in0=srcт[:, 0:NK],
                        s0=s0, s1=wrapv_sb[:, 2 * r : 2 * r + 1], imm2=MAGIC,
                    )
                    nc.vector._custom_dve(
                        frac_op, out=wr[:, NK : NK + NQ],
                        in0=srcт[:, NK : NK + NQ],
                        s0=s0, s1=wrapv_sb[:, 2 * r + 1 : 2 * r + 2], imm2=MAGIC,
                    )
                    wr_t[r] = wr

                emit_wraps(0)
                nc.scalar.activation(dup[:, :], pkq_t[:, :], AF.Identity)

                # linear term first in each psum group (PE runs these while
                # waiting for the first sine features)
                for kt in range(5):
                    ks = slice(kt * 128, kt * 128 + KT_SIZES[kt])
                    nc.tensor.matmul(
                        out=scores[kt][:], lhsT=dup[0:CO, ks], rhs=awfbc_sb[:],
                        start=True, stop=False, skip_group_check=True,
                    )
                    nc.tensor.matmul(
                        out=scores[kt][:], lhsT=awfbc_sb[:, 0 : KT_SIZES[kt]],
                        rhs=dup[0:CO, NK : NK + NQ],
                        start=False, stop=False, skip_group_check=True,
                    )

                for r in range(R):
                    fr = fp.tile([128, NK + NQ], f16, tag="fr")
                    nc.scalar.activation(fr[:], wr_t[r][:], AF.Sin, scale=TWO_PI)
                    fr_t[r] = fr
                    if r + 1 < R:
                        emit_wraps(r + 1)
                    gr = gp.tile([128, NQ], f16, tag="gr")
                    if r % 2 == 0:
                        nc.scalar.activation(
                            gr[:], fr[:, NK : NK + NQ], AF.Identity,
                            scale=qscale_sb[:, r : r + 1],
                        )
                    else:
                        nc.vector.tensor_scalar_mul(
                            out=gr[:], in0=fr[:, NK : NK + NQ],
                            scalar1=qscale_sb[:, r : r + 1],
                        )
                    for kt in range(5):
                        ks = slice(kt * 128, kt * 128 + KT_SIZES[kt])
                        nc.tensor.matmul(
                            out=scores[kt][:], lhsT=fr[:, ks], rhs=gr[:],
                            start=False, stop=(r == R - 1),
                            skip_group_check=True,
                        )

                # attn_t = tanh(0.5*scores + bfv); sigmoid = 0.5+0.5*attn_t
                for kt in range(5):
                    nc.scalar.activation(
                        attn_sb[kt][:], scores[kt][:], AF.Tanh,
                        scale=0.5, bias=bfv_sb[: KT_SIZES[kt]],
                    )

                # out = 0.5*vsum + (0.5*value) @ attn_t  (0.5 folded on host;
                # the output matmuls reuse scores[ct]'s psum bank - its
                # sigmoid has already read it)
                for ct in range(2):
                    po = scores[ct]
                    for kt in range(5):
                        nc.tensor.matmul(
                            out=po[:],
                            lhsT=vt_sb[kt][:, ct * 128 : (ct + 1) * 128],
                            rhs=attn_sb[kt][:],
                            start=(kt == 0), stop=(kt == 4),
                            skip_group_check=True,
                        )
                    nc.vector.tensor_scalar_add(
                        out=osb[ct][:], in0=po[:], scalar1=vs_sb[:, ct : ct + 1]
                    )
                    (nc.sync if ct == 0 else nc.scalar).dma_start(
                        out=outd.ap()[ct * 128 : (ct + 1) * 128, :], in_=osb[ct][:]
                    )
    nc.finalize()
    return nc


def _prep_in_maps(key, query, value, Wk, bk, Wq, bq, wf, bf):
    f32, f16 = np.float32, np.float16
    key = np.ascontiguousarray(key, f32).reshape(B, C, NK)
    query = np.ascontiguousarray(query, f32).reshape(B, C, HW, HW)
    value = np.ascontiguousarray(value, f32).reshape(B, C, NK)
    Wk = np.asarray(Wk, f32)
    Wq = np.asarray(Wq, f32)
    wf = np.asarray(wf, f32)
    bk = np.asarray(bk, f32)
    bq = np.asarray(bq, f32)
    bf = np.float32(bf)

    wkt2 = np.concatenate([Wk.T, Wk.T], axis=1)  # (256, 128)
    wqt2 = np.concatenate([Wq.T, Wq.T], axis=1)  # (256, 128)
    wkq = np.ascontiguousarray(np.concatenate([wkt2, wqt2], axis=1)).astype(f16)

    # wrap phase/bias vectors, in frac (turns) units. Feature rows:
    # p < 64: c = p, k-side sin / q-side cos;  p >= 64: c = p-64, k-side cos
    # / q-side sin.  C1 = (om*b? + phase)/2pi.
    wrapv = np.zeros((128, 2 * R), f32)
    qsc = np.zeros((128, R), f32)
    for r in range(R):
        om = np.float32(OM[r])
        wrapv[:64, 2 * r] = om * bk / TWO_PI
        wrapv[64:, 2 * r] = om * bk / TWO_PI + 0.25
        wrapv[:64, 2 * r + 1] = om * bq / TWO_PI + 0.25
        wrapv[64:, 2 * r + 1] = om * bq / TWO_PI
        qsc[:64, r] = BB[r] * wf
        qsc[64:, r] = BB[r] * wf
    awfbc = np.ascontiguousarray(
        np.broadcast_to((A_LIN * wf)[:, None], (CO, NQ))
    ).astype(f16)
    # linear term uses raw k_/q_ (biases folded here); sigmoid-as-tanh halves
    bf_eff = bf + A_LIN * float(wf @ (bk + bq))
    bfv = np.full((128, 1), 0.5 * bf_eff, f32)

    key16 = key.astype(f16)
    query16 = query.astype(f16)
    common = {"wkq": wkq, "wrapv": wrapv, "qscale": qsc, "awfbc": awfbc, "bfv": bfv}
    in_maps = []
    for i in range(8):
        b, h = i // 2, i % 2
        qs = np.ascontiguousarray(
            query16[b, :, h * 12 : (h + 1) * 12, :]
        ).reshape(C, NQ)
        valt05 = np.ascontiguousarray((0.5 * value[b]).T).astype(f16)  # (576, 256)
        vsum = 0.5 * value[b].sum(axis=1)  # (256,)
        vs2 = np.zeros((128, 2), f32)
        vs2[:, 0] = vsum[:128]
        vs2[:, 1] = vsum[128:]
        m = {
            "kqin": np.ascontiguousarray(
                np.concatenate([key16[b], qs], axis=1)
            ),
            "valtin": valt05,
            "vsum05": vs2,
        }
        m.update(common)
        in_maps.append(m)
    return in_maps


def run(trace=False, **inputs):
    from concourse.bass_utils import run_bass_kernel_spmd

    inputs.pop("mode", None)
    inputs.pop("chunk", None)
    if "nc" not in _cache:
        _cache["nc"] = _build()
    nc = _cache["nc"]
    in_maps = _prep_in_maps(**inputs)
    res = run_bass_kernel_spmd(nc, in_maps, core_ids=list(range(8)), trace=trace)
    out = np.empty((B, C, HW, HW), np.float32)
    for i in range(8):
        b, h = i // 2, i % 2
        out[b, :, h * 12 : (h + 1) * 12, :] = res.results[i]["out"].reshape(C, 12, HW)
    return out, res


def kernel(**inputs):
    out, _ = run(**inputs)
    return out


# revision 16
# speedup vs baseline: 1.0071x; 1.0071x over previous
"""Additive attention (B=4, C=256, CO=64, H=W=24) on 8 TRN2 NeuronCores.

Sharding: core i handles batch b = i // 2 and Nq-half h = i % 2 (rows
12h..12h+12 of the 24x24 query grid). Each core produces a complete
(256, 288) slice of the output; no collectives are needed.

Algorithm (Fourier-factorized additive attention): the score tensor
  scores[k, q] = sum_c wf_c * tanh(k_c[k] + q_c[q])
is O(Nk*Nq*CO) elementwise work if computed directly (the tanh alone is
~69us/core on the ACT engine). Instead approximate
  tanh(x) ~= a*x + sum_r b_r sin(om_r x)
(free-frequency least-squares fit, weighted by the N(0,2) density of
x = k_c + q_c; R=5 gives weighted-RMS error 9.3e-4) and use
  sin(om(k+q)) = sin(om k)cos(om q) + cos(om k)sin(om q),
which factorizes scores into a rank-(2R*CO + 2) matmul:
  scores = F(k)^T G(q) + a*(Ak[k] + Aq[q]),
with F/G = {sin,cos}(om_r * .) feature maps over the 64 channels. The
O(N^2 C) tanh becomes an O(N^2 * 2R*C) PE matmul plus O(N*C*R)
elementwise sin work - engines: PE ~8us, ACT ~7us, DVE ~6us per core.

Range reduction for sin: a custom DVE op (FRAC_SHIFT_ANT, registered at
runtime) computes f = y - round(y) with y = x*(om/2pi) + phase/2pi via
the fp32 magic-constant rounding trick; ACT then evaluates
sin(2pi * f), arg range exactly [-pi, pi] (the ACT Sin table diverges
beyond ~|3.5| rad). cos rides the same op via phase=0.25.

sigmoid(s) is computed as 0.5 + 0.5*tanh(0.5 s) (Sin and Tanh share
one ACT table -> no table reloads); the 0.5 offset becomes a
0.5*rowsum(value) correction added at the output copy, and the 0.5
factor folds into the transposed-value tiles.

Measured: ~? us exec (neuron-profile), predicted rel err ~5e-4.
"""

import numpy as np

B, C, CO, HW = 4, 256, 64, 24
NK = 576
NQ = 288  # per-core query count (half of 576)
KT_SIZES = [128, 128, 128, 128, 64]

# tanh(x) ~= A_LIN*x + sum_r BB[r]*sin(OM[r]*x); weighted LSQ fit on N(0,2)
# R=4: weighted-RMS 2.4e-3, end-to-end rel err ~1.2e-3 (R=5 alt: 0.18780 /
# [0.589796,1.188114,1.868618,2.723939,3.824876] /
# [0.553768,0.196597,0.080735,0.02663,0.006279] -> 5.3e-4)
A_LIN = 0.18960
OM = [0.595782, 1.259669, 2.109728, 3.210177]
BB = [0.561325, 0.210306, 0.069877, 0.016487]
R = len(OM)
TWO_PI = float(2.0 * np.pi)
MAGIC = 12582912.0  # 3 * 2^22: fp32 round-to-nearest-integer constant

_cache = {}


def _register_frac_op():
    """Register the FRAC_SHIFT_ANT custom DVE op (idempotent):
    out = y - round(y), y = in0*s0 + s1  (all fp32; round via +/-MAGIC).
    """
    import concourse.dve_ops as dve_ops
    from concourse.dve_spec import Spec, Src0, C0, C1, C2, lower
    from concourse.dve_uop import DveOpSpec

    for op in dve_ops.OPS:
        if op.name == "FRAC_SHIFT_ANT":
            return op

    y = Src0 * C0 + C1
    n = (y + C2) - C2
    spec = Spec(
        body=y - n,
        reference=lambda in0, in1, s0, s1, imm2: (
            lambda yy: yy
            - ((yy + np.float32(imm2)).astype(np.float32) - np.float32(imm2))
        )((np.float32(in0) * np.float32(s0) + np.float32(s1)).astype(np.float32)),
    )
    opcode = dve_ops._CUSTOM_DVE_ROW_BASE + len(dve_ops.OPS)
    shas = {}
    for ver in ("v3", "v4"):
        shas[ver] = DveOpSpec(
            name="FRAC_SHIFT_ANT", opcode=opcode, uops=lower(spec, ver=ver),
            rd1_en=False,
        ).sha(ver)
    op = dve_ops.DveOp("FRAC_SHIFT_ANT", spec, subdim=False, uops_sha=shas)
    dve_ops.OPS.append(op)
    dve_ops.CUSTOM_DVE_SPECS[op.name] = op.spec
    dve_ops._SUB_OPCODE_FOR_NAME[op.name] = opcode
    return op


def _build():
    import concourse.bacc as bacc
    import concourse.mybir as mybir
    from concourse.tile import TileContext

    frac_op = _register_frac_op()

    f32 = mybir.dt.float32
    f16 = mybir.dt.float16
    AF = mybir.ActivationFunctionType

    nc = bacc.Bacc("TRN2", target_bir_lowering=False, debug=False, num_devices=8)
    with TileContext(nc) as tc:
        kqin = nc.dram_tensor("kqin", [C, NK + NQ], f16, kind="ExternalInput")
        valtin = nc.dram_tensor("valtin", [NK, C], f16, kind="ExternalInput")
        wkq = nc.dram_tensor("wkq", [C, 256], f16, kind="ExternalInput")
        # uv: cols 0:NQ = ubc (u[cin] bcast over q), NQ:NQ+128 = v2bc
        uvin = nc.dram_tensor("uvin", [C, NQ + 128], f16, kind="ExternalInput")
        # biasrow: rank-2 bias add into pkq. cols 0:864 = rhs ([ones|0;0|ones]
        # masked), 864:992 = lhsT ([bk-dup; bq-dup])
        biasrow = nc.dram_tensor("biasrow", [2, NK + NQ + 128], f16, kind="ExternalInput")
        # cpack f32: cols 0:R = qscale (+-b_r wf dup), R = phase vec (1/8,
        # 3/8 turns), R+1 = bfv, R+2:R+4 = vsum05 per ct
        cpack = nc.dram_tensor("cpack", [128, R + 4], f32, kind="ExternalInput")
        outd = nc.dram_tensor("out", [C, NQ], f32, kind="ExternalOutput")

        with (
            tc.tile_pool(name="consts", bufs=1) as consts,
            tc.tile_pool(name="inp", bufs=1) as inp,
            tc.tile_pool(name="work", bufs=1) as work,
            tc.tile_pool(name="wp", bufs=2) as wp,
            tc.tile_pool(name="fp", bufs=2) as fp,
            tc.tile_pool(name="gp", bufs=2) as gp,
            tc.tile_pool(name="psum", bufs=1, space="PSUM") as psum,
        ):
            kq_sb = [inp.tile([128, NK + NQ], f16, tag=f"kq{t}", name=f"kq{t}") for t in range(2)]
            vt_sb = [
                inp.tile([KT_SIZES[kt], C], f16, tag=f"vt{kt}", name=f"vt{kt}")
                for kt in range(5)
            ]
            wkq_sb = [consts.tile([128, 256], f16, tag=f"wkq{t}", name=f"wkq{t}") for t in range(2)]
            uv_sb = [consts.tile([128, NQ + 128], f16, tag=f"uv{t}", name=f"uv{t}") for t in range(2)]
            br_sb = consts.tile([2, NK + NQ + 128], f16, tag="br")
            cp_sb = consts.tile([128, R + 4], f32, tag="cp")
            scr = consts.tile([128, 1], f32, tag="scr")
            warm = consts.tile([128, 512], f16, tag="warm")

            # DMA issue order matters per queue (SP / ACT / Pool streams).
            nc.sync.dma_start(out=kq_sb[0][:], in_=kqin.ap()[0:128, :])
            nc.scalar.dma_start(out=wkq_sb[0][:], in_=wkq.ap()[0:128, :])
            nc.gpsimd.dma_start(out=kq_sb[1][:], in_=kqin.ap()[128:256, :])
            nc.scalar.dma_start(out=wkq_sb[1][:], in_=wkq.ap()[128:256, :])
            nc.sync.dma_start(out=cp_sb[:], in_=cpack.ap())
            nc.sync.dma_start(out=br_sb[:], in_=biasrow.ap())
            nc.sync.dma_start(out=uv_sb[0][:], in_=uvin.ap()[0:128, :])
            nc.scalar.dma_start(out=uv_sb[1][:], in_=uvin.ap()[128:256, :])
            # warmup: Silu pins the silu_and_others ACT table (sin+tanh+silu
            # in one table -> no mid-kernel ACT_TABLE_LOADs)
            nc.vector.memset(scr[:], 0.0)
            nc.scalar.activation(scr[:], scr[:], AF.Silu)
            nc.vector.memset(warm[:], 0.0)
            for kt in range(5):
                nc.gpsimd.dma_start(
                    out=vt_sb[kt][:],
                    in_=valtin.ap()[kt * 128 : kt * 128 + KT_SIZES[kt], :],
                )

            attn_sb = [
                work.tile([KT_SIZES[kt], NQ], f16, tag=f"attn{kt}", name=f"attn{kt}")
                for kt in range(5)
            ]
            osb = [work.tile([128, NQ], f32, tag=f"osb{t}", name=f"osb{t}") for t in range(2)]

            pkq_t = psum.tile([128, NK + NQ], f32, tag="pkq")
            scores = [
                psum.tile([KT_SIZES[kt], NQ], f32, tag=f"sc{kt}", name=f"sc{kt}")
                for kt in range(5)
            ]

            # PE p-state warmup in the DMA shadow (throwaway group in pkq_t)
            for i in range(8):
                nc.tensor.matmul(
                    out=pkq_t[:, 0:512], lhsT=warm[:, 0:128], rhs=warm[:],
                    start=(i == 0), stop=(i == 7),
                )

            # k_/q_ = [W|W]^T @ (key|qry) + rank-2 bias add -> dup rows psum
            for c0, c1 in ((0, 512), (512, NK), (NK, NK + NQ)):
                col = 0 if c1 <= NK else 128
                for ct in range(2):
                    nc.tensor.matmul(
                        out=pkq_t[:, c0:c1],
                        lhsT=wkq_sb[ct][:, col : col + 128],
                        rhs=kq_sb[ct][:, c0:c1],
                        start=(ct == 0), stop=False, skip_group_check=True,
                    )
                nc.tensor.matmul(
                    out=pkq_t[:, c0:c1],
                    lhsT=br_sb[:, NK + NQ : NK + NQ + 128],
                    rhs=br_sb[:, c0:c1],
                    start=False, stop=True, skip_group_check=True,
                )

            # linear term, from the raw inputs (PE-ready at DMA land):
            # scores[k,q] += u^T key (bcast q) + v2^T qry (bcast k)
            for kt in range(5):
                ks = slice(kt * 128, kt * 128 + KT_SIZES[kt])
                for ct in range(2):
                    nc.tensor.matmul(
                        out=scores[kt][:], lhsT=kq_sb[ct][:, ks],
                        rhs=uv_sb[ct][:, 0:NQ],
                        start=(ct == 0), stop=False, skip_group_check=True,
                    )
                for ct in range(2):
                    nc.tensor.matmul(
                        out=scores[kt][:],
                        lhsT=uv_sb[ct][:, NQ : NQ + KT_SIZES[kt]],
                        rhs=kq_sb[ct][:, NK : NK + NQ],
                        start=False, stop=False, skip_group_check=True,
                    )

            # Fourier features: one FRAC_SHIFT per r over [k|q] (the pi/4
            # phase trick makes both sides share one phase vector; the sign
            # lands in qscale), one Sin ACT per r, one q-scale per r.
            wr_t = [None] * R

            def emit_wrap(r):
                wr = wp.tile([128, NK + NQ], f32, tag="wr")
                nc.vector._custom_dve(
                    frac_op, out=wr[:], in0=pkq_t[:],
                    s0=float(OM[r] / TWO_PI), s1=cp_sb[:, R : R + 1], imm2=MAGIC,
                )
                wr_t[r] = wr

            emit_wrap(0)
            for r in range(R):
                fr = fp.tile([128, NK + NQ], f16, tag="fr")
                nc.scalar.activation(fr[:], wr_t[r][:], AF.Sin, scale=TWO_PI)
                if r + 1 < R:
                    emit_wrap(r + 1)
                gr = gp.tile([128, NQ], f16, tag="gr")
                if r % 2 == 0:
                    nc.scalar.activation(
                        gr[:], fr[:, NK : NK + NQ], AF.Identity,
                        scale=cp_sb[:, r : r + 1],
                    )
                else:
                    nc.vector.tensor_scalar_mul(
                        out=gr[:], in0=fr[:, NK : NK + NQ],
                        scalar1=cp_sb[:, r : r + 1],
                    )
                for kt in range(5):
                    ks = slice(kt * 128, kt * 128 + KT_SIZES[kt])
                    nc.tensor.matmul(
                        out=scores[kt][:], lhsT=fr[:, ks], rhs=gr[:],
                        start=False, stop=(r == R - 1),
                        skip_group_check=True,
                    )

            # attn_t = tanh(0.5*scores + bfv); sigmoid = 0.5 + 0.5*attn_t
            for kt in range(5):
                nc.scalar.activation(
                    attn_sb[kt][:], scores[kt][:], AF.Tanh,
                    scale=0.5, bias=cp_sb[: KT_SIZES[kt], R + 1 : R + 2],
                )

            # out = 0.5*vsum + (0.5*value) @ attn_t  (0.5 folded on host; the
            # output matmuls reuse scores[ct]'s psum bank)
            for ct in range(2):
                po = scores[ct]
                for kt in range(5):
                    nc.tensor.matmul(
                        out=po[:],
                        lhsT=vt_sb[kt][:, ct * 128 : (ct + 1) * 128],
                        rhs=attn_sb[kt][:],
                        start=(kt == 0), stop=(kt == 4),
                        skip_group_check=True,
                    )
                nc.vector.tensor_scalar_add(
                    out=osb[ct][:], in0=po[:],
                    scalar1=cp_sb[:, R + 2 + ct : R + 3 + ct],
                )
                (nc.sync if ct == 0 else nc.scalar).dma_start(
                    out=outd.ap()[ct * 128 : (ct + 1) * 128, :], in_=osb[ct][:]
                )
    nc.finalize()
    return nc


def _prep_in_maps(key, query, value, Wk, bk, Wq, bq, wf, bf):
    f32, f16 = np.float32, np.float16
    key = np.ascontiguousarray(key, f32).reshape(B, C, NK)
    query = np.ascontiguousarray(query, f32).reshape(B, C, HW, HW)
    value = np.ascontiguousarray(value, f32).reshape(B, C, NK)
    Wk = np.asarray(Wk, f32)
    Wq = np.asarray(Wq, f32)
    wf = np.asarray(wf, f32)
    bk = np.asarray(bk, f32)
    bq = np.asarray(bq, f32)
    bf = np.float32(bf)

    wkt2 = np.concatenate([Wk.T, Wk.T], axis=1)  # (256, 128)
    wqt2 = np.concatenate([Wq.T, Wq.T], axis=1)
    wkq = np.ascontiguousarray(np.concatenate([wkt2, wqt2], axis=1)).astype(f16)

    # linear-term vectors (biases folded into bf_eff)
    u = (A_LIN * wf) @ Wk   # (256,)
    v2 = (A_LIN * wf) @ Wq
    uv = np.zeros((C, NQ + 128), f32)
    uv[:, 0:NQ] = u[:, None]
    uv[:, NQ : NQ + 128] = v2[:, None]
    uv = np.ascontiguousarray(uv).astype(f16)

    # rank-2 bias add into the duplicated k_/q_ psum
    brow = np.zeros((2, NK + NQ + 128), f32)
    brow[0, 0:NK] = 1.0
    brow[1, NK : NK + NQ] = 1.0
    brow[0, NK + NQ :] = np.tile(bk, 2)
    brow[1, NK + NQ :] = np.tile(bq, 2)
    brow = brow.astype(f16)

    cpk = np.zeros((128, R + 4), f32)
    for r in range(R):
        cpk[:64, r] = BB[r] * wf      # + sign: sin(x+pi/4) rows
        cpk[64:, r] = -BB[r] * wf     # - sign: sin(x+3pi/4)=cos(x+pi/4) rows
    cpk[:64, R] = 0.125               # phase, in turns
    cpk[64:, R] = 0.375
    bf_eff = bf + A_LIN * float(wf @ (bk + bq))
    cpk[:, R + 1] = 0.5 * bf_eff

    key16 = key.astype(f16)
    query16 = query.astype(f16)
    common = {"wkq": wkq, "uvin": uv, "biasrow": brow}
    in_maps = []
    for i in range(8):
        b, h = i // 2, i % 2
        qs = np.ascontiguousarray(
            query16[b, :, h * 12 : (h + 1) * 12, :]
        ).reshape(C, NQ)
        valt05 = np.ascontiguousarray((0.5 * value[b]).T).astype(f16)
        vsum = 0.5 * value[b].sum(axis=1)
        cp = cpk.copy()
        cp[:, R + 2] = vsum[:128]
        cp[:, R + 3] = vsum[128:]
        m = {
            "kqin": np.ascontiguousarray(np.concatenate([key16[b], qs], axis=1)),
            "valtin": valt05,
            "cpack": cp,
        }
        m.update(common)
        in_maps.append(m)
    return in_maps


def run(trace=False, **inputs):
    from concourse.bass_utils import run_bass_kernel_spmd

    inputs.pop("mode", None)
    inputs.pop("chunk", None)
    if "nc" not in _cache:
        _cache["nc"] = _build()
    nc = _cache["nc"]
    in_maps = _prep_in_maps(**inputs)
    res = run_bass_kernel_spmd(nc, in_maps, core_ids=list(range(8)), trace=trace)
    out = np.empty((B, C, HW, HW), np.float32)
    for i in range(8):
        b, h = i // 2, i % 2
        out[b, :, h * 12 : (h + 1) * 12, :] = res.results[i]["out"].reshape(C, 12, HW)
    return out, res


def kernel(**inputs):
    out, _ = run(**inputs)
    return out


# revision 18
# speedup vs baseline: 1.0557x; 1.0482x over previous
"""Additive attention (B=4, C=256, CO=64, H=W=24) on 8 TRN2 NeuronCores.

Sharding: core i handles batch b = i // 2 and Nq-half h = i % 2 (rows
12h..12h+12 of the 24x24 query grid). Each core produces a complete
(256, 288) slice of the output; no collectives are needed.

Algorithm (Fourier-factorized additive attention): the score tensor
  scores[k, q] = sum_c wf_c * tanh(k_c[k] + q_c[q])
is O(Nk*Nq*CO) elementwise work if computed directly (the tanh alone is
~69us/core on the ACT engine). Instead approximate
  tanh(x) ~= a*x + sum_r b_r sin(om_r x)
(free-frequency least-squares fit, weighted by the N(0,2) density of
x = k_c + q_c; R=5 gives weighted-RMS error 9.3e-4) and use
  sin(om(k+q)) = sin(om k)cos(om q) + cos(om k)sin(om q),
which factorizes scores into a rank-(2R*CO + 2) matmul:
  scores = F(k)^T G(q) + a*(Ak[k] + Aq[q]),
with F/G = {sin,cos}(om_r * .) feature maps over the 64 channels. The
O(N^2 C) tanh becomes an O(N^2 * 2R*C) PE matmul plus O(N*C*R)
elementwise sin work - engines: PE ~8us, ACT ~7us, DVE ~6us per core.

Range reduction for sin: a custom DVE op (FRAC_SHIFT_ANT, registered at
runtime) computes f = y - round(y) with y = x*(om/2pi) + phase/2pi via
the fp32 magic-constant rounding trick; ACT then evaluates
sin(2pi * f), arg range exactly [-pi, pi] (the ACT Sin table diverges
beyond ~|3.5| rad). cos rides the same op via phase=0.25.

sigmoid(s) is computed as 0.5 + 0.5*tanh(0.5 s) (Sin and Tanh share
one ACT table -> no table reloads); the 0.5 offset becomes a
0.5*rowsum(value) correction added at the output copy, and the 0.5
factor folds into the transposed-value tiles.

Measured: ~? us exec (neuron-profile), predicted rel err ~5e-4.
"""

import numpy as np

B, C, CO, HW = 4, 256, 64, 24
NK = 576
NQ = 288  # per-core query count (half of 576)
KT_SIZES = [128, 128, 128, 128, 64]

# tanh(x) ~= A_LIN*x + sum_r BB[r]*sin(OM[r]*x); weighted LSQ fit on N(0,2)
# R=4: weighted-RMS 2.4e-3, end-to-end rel err ~1.2e-3 (R=5 alt: 0.18780 /
# [0.589796,1.188114,1.868618,2.723939,3.824876] /
# [0.553768,0.196597,0.080735,0.02663,0.006279] -> 5.3e-4)
A_LIN = 0.18960
OM = [0.595782, 1.259669, 2.109728, 3.210177]
BB = [0.561325, 0.210306, 0.069877, 0.016487]
R = len(OM)
TWO_PI = float(2.0 * np.pi)
MAGIC = 12582912.0  # 3 * 2^22: fp32 round-to-nearest-integer constant

_cache = {}


def _register_frac_op():
    """Register the FRAC_SHIFT_ANT custom DVE op (idempotent):
    out = y - round(y), y = in0*s0 + s1  (all fp32; round via +/-MAGIC).
    """
    import concourse.dve_ops as dve_ops
    from concourse.dve_spec import Spec, Src0, C0, C1, C2, lower
    from concourse.dve_uop import DveOpSpec

    for op in dve_ops.OPS:
        if op.name == "FRAC_SHIFT_ANT":
            return op

    y = Src0 * C0 + C1
    n = (y + C2) - C2
    spec = Spec(
        body=y - n,
        reference=lambda in0, in1, s0, s1, imm2: (
            lambda yy: yy
            - ((yy + np.float32(imm2)).astype(np.float32) - np.float32(imm2))
        )((np.float32(in0) * np.float32(s0) + np.float32(s1)).astype(np.float32)),
    )
    opcode = dve_ops._CUSTOM_DVE_ROW_BASE + len(dve_ops.OPS)
    shas = {}
    for ver in ("v3", "v4"):
        shas[ver] = DveOpSpec(
            name="FRAC_SHIFT_ANT", opcode=opcode, uops=lower(spec, ver=ver),
            rd1_en=False,
        ).sha(ver)
    op = dve_ops.DveOp("FRAC_SHIFT_ANT", spec, subdim=False, uops_sha=shas)
    dve_ops.OPS.append(op)
    dve_ops.CUSTOM_DVE_SPECS[op.name] = op.spec
    dve_ops._SUB_OPCODE_FOR_NAME[op.name] = opcode
    return op


def _build():
    import concourse.bacc as bacc
    import concourse.mybir as mybir
    from concourse.tile import TileContext

    frac_op = _register_frac_op()

    f32 = mybir.dt.float32
    f16 = mybir.dt.float16
    AF = mybir.ActivationFunctionType

    nc = bacc.Bacc("TRN2", target_bir_lowering=False, debug=False, num_devices=8)
    with TileContext(nc) as tc:
        kqin = nc.dram_tensor("kqin", [C, NK + NQ], f16, kind="ExternalInput")
        valtin = nc.dram_tensor("valtin", [NK + 1, C], f16, kind="ExternalInput")
        wkq = nc.dram_tensor("wkq", [C, 256], f16, kind="ExternalInput")
        # uv: cols 0:NQ = ubc (u[cin] bcast over q), NQ:NQ+128 = v2bc
        uvin = nc.dram_tensor("uvin", [C, NQ + 128], f16, kind="ExternalInput")
        # biasrow: rank-2 bias add into pkq. cols 0:864 = rhs ([ones|0;0|ones]
        # masked), 864:992 = lhsT ([bk-dup; bq-dup])
        biasrow = nc.dram_tensor("biasrow", [2, NK + NQ + 128], f16, kind="ExternalInput")
        # cpack f32: cols 0:R = qscale (+-b_r wf dup), R = phase vec (1/8,
        # 3/8 turns), R+1 = bfv, R+2:R+4 = vsum05 per ct
        cpack = nc.dram_tensor("cpack", [128, R + 4], f32, kind="ExternalInput")
        outd = nc.dram_tensor("out", [C, NQ], f32, kind="ExternalOutput")

        with (
            tc.tile_pool(name="consts", bufs=1) as consts,
            tc.tile_pool(name="inp", bufs=1) as inp,
            tc.tile_pool(name="work", bufs=1) as work,
            tc.tile_pool(name="wp", bufs=2) as wp,
            tc.tile_pool(name="fp", bufs=2) as fp,
            tc.tile_pool(name="gp", bufs=2) as gp,
            tc.tile_pool(name="psum", bufs=1, space="PSUM") as psum,
        ):
            kq_sb = [inp.tile([128, NK + NQ], f16, tag=f"kq{t}", name=f"kq{t}") for t in range(2)]
            vt_sb = [
                inp.tile([KT_SIZES[kt] + (kt == 4), C], f16, tag=f"vt{kt}", name=f"vt{kt}")
                for kt in range(5)
            ]
            wkq_sb = [consts.tile([128, 256], f16, tag=f"wkq{t}", name=f"wkq{t}") for t in range(2)]
            uv_sb = [consts.tile([128, NQ + 128], f16, tag=f"uv{t}", name=f"uv{t}") for t in range(2)]
            br_sb = consts.tile([2, NK + NQ + 128], f16, tag="br")
            cp_sb = consts.tile([128, R + 4], f32, tag="cp")
            scr = consts.tile([128, 1], f32, tag="scr")
            warm = consts.tile([128, 512], f16, tag="warm")

            # DMA issue order matters per queue (SP / ACT / Pool streams).
            nc.sync.dma_start(out=kq_sb[0][:], in_=kqin.ap()[0:128, :])
            nc.scalar.dma_start(out=wkq_sb[0][:], in_=wkq.ap()[0:128, :])
            nc.gpsimd.dma_start(out=kq_sb[1][:], in_=kqin.ap()[128:256, :])
            nc.scalar.dma_start(out=wkq_sb[1][:], in_=wkq.ap()[128:256, :])
            nc.sync.dma_start(out=cp_sb[:], in_=cpack.ap())
            nc.sync.dma_start(out=br_sb[:], in_=biasrow.ap())
            nc.sync.dma_start(out=uv_sb[0][:], in_=uvin.ap()[0:128, :])
            nc.scalar.dma_start(out=uv_sb[1][:], in_=uvin.ap()[128:256, :])
            # warmup: Silu pins the silu_and_others ACT table (sin+tanh+silu
            # in one table -> no mid-kernel ACT_TABLE_LOADs)
            nc.vector.memset(scr[:], 0.0)
            nc.scalar.activation(scr[:], scr[:], AF.Silu)
            nc.vector.memset(warm[:], 0.0)
            for kt in range(5):
                nc.gpsimd.dma_start(
                    out=vt_sb[kt][:],
                    in_=valtin.ap()[kt * 128 : kt * 128 + KT_SIZES[kt] + (kt == 4), :],
                )

            # attn4/vt4 carry a 65th row (ones / 0.5*vsum) so the value
            # matmul adds the sigmoid 0.5-offset correction for free
            attn_sb = [
                work.tile([KT_SIZES[kt] + (kt == 4), NQ], f16, tag=f"attn{kt}", name=f"attn{kt}")
                for kt in range(5)
            ]
            nc.vector.memset(attn_sb[4][64:65, :], 1.0)
            osb = [work.tile([128, NQ], f32, tag=f"osb{t}", name=f"osb{t}") for t in range(2)]

            pkq_t = psum.tile([128, NK + NQ], f32, tag="pkq")
            scores = [
                psum.tile([KT_SIZES[kt], NQ], f32, tag=f"sc{kt}", name=f"sc{kt}")
                for kt in range(5)
            ]

            # PE p-state warmup in the DMA shadow (throwaway group in pkq_t)
            for i in range(8):
                nc.tensor.matmul(
                    out=pkq_t[:, 0:512], lhsT=warm[:, 0:128], rhs=warm[:],
                    start=(i == 0), stop=(i == 7),
                )

            # k_/q_ = [W|W]^T @ (key|qry) + rank-2 bias add -> dup rows psum
            for c0, c1 in ((0, 512), (512, NK), (NK, NK + NQ)):
                col = 0 if c1 <= NK else 128
                for ct in range(2):
                    nc.tensor.matmul(
                        out=pkq_t[:, c0:c1],
                        lhsT=wkq_sb[ct][:, col : col + 128],
                        rhs=kq_sb[ct][:, c0:c1],
                        start=(ct == 0), stop=False, skip_group_check=True,
                    )
                nc.tensor.matmul(
                    out=pkq_t[:, c0:c1],
                    lhsT=br_sb[:, NK + NQ : NK + NQ + 128],
                    rhs=br_sb[:, c0:c1],
                    start=False, stop=True, skip_group_check=True,
                )

            # linear term, from the raw inputs (PE-ready at DMA land):
            # scores[k,q] += u^T key (bcast q) + v2^T qry (bcast k)
            for kt in range(5):
                ks = slice(kt * 128, kt * 128 + KT_SIZES[kt])
                for ct in range(2):
                    nc.tensor.matmul(
                        out=scores[kt][:], lhsT=kq_sb[ct][:, ks],
                        rhs=uv_sb[ct][:, 0:NQ],
                        start=(ct == 0), stop=False, skip_group_check=True,
                    )
                for ct in range(2):
                    nc.tensor.matmul(
                        out=scores[kt][:],
                        lhsT=uv_sb[ct][:, NQ : NQ + KT_SIZES[kt]],
                        rhs=kq_sb[ct][:, NK : NK + NQ],
                        start=False, stop=False, skip_group_check=True,
                    )

            # Fourier features: one FRAC_SHIFT per r over [k|q] (the pi/4
            # phase trick makes both sides share one phase vector; the sign
            # lands in qscale), one Sin ACT per r, one q-scale per r.
            wr_t = [None] * R

            def emit_wrap(r):
                wr = wp.tile([128, NK + NQ], f32, tag="wr")
                nc.vector._custom_dve(
                    frac_op, out=wr[:], in0=pkq_t[:],
                    s0=float(OM[r] / TWO_PI), s1=cp_sb[:, R : R + 1], imm2=MAGIC,
                )
                wr_t[r] = wr

            emit_wrap(0)
            fr_t = [None] * R

            def emit_tail(r):
                # q-feature scale + the 5 score matmuls for round r
                fr = fr_t[r]
                gr = gp.tile([128, NQ], f16, tag="gr")
                if r % 2 == 0:
                    nc.scalar.activation(
                        gr[:], fr[:, NK : NK + NQ], AF.Identity,
                        scale=cp_sb[:, r : r + 1],
                    )
                else:
                    nc.vector.tensor_scalar_mul(
                        out=gr[:], in0=fr[:, NK : NK + NQ],
                        scalar1=cp_sb[:, r : r + 1],
                    )
                for kt in range(5):
                    ks = slice(kt * 128, kt * 128 + KT_SIZES[kt])
                    nc.tensor.matmul(
                        out=scores[kt][:], lhsT=fr[:, ks], rhs=gr[:],
                        start=False, stop=(r == R - 1),
                        skip_group_check=True,
                    )

            for r in range(R):
                fr = fp.tile([128, NK + NQ], f16, tag="fr")
                nc.scalar.activation(fr[:], wr_t[r][:], AF.Sin, scale=TWO_PI)
                fr_t[r] = fr
                if r + 1 < R:
                    emit_wrap(r + 1)
                if r >= 1:
                    emit_tail(r - 1)
            emit_tail(R - 1)

            # attn_t = tanh(0.5*scores + bfv); sigmoid = 0.5 + 0.5*attn_t
            for kt in range(5):
                nc.scalar.activation(
                    attn_sb[kt][: KT_SIZES[kt], :], scores[kt][:], AF.Tanh,
                    scale=0.5, bias=cp_sb[: KT_SIZES[kt], R + 1 : R + 2],
                )

            # out = (0.5*value | 0.5*vsum) @ (attn_t | ones): the 65th row of
            # the kt=4 pair adds the 0.5*vsum offset; output DMAs straight
            # from psum (reusing scores[ct]'s bank)
            for ct in range(2):
                po = scores[ct]
                for kt in range(5):
                    nc.tensor.matmul(
                        out=po[:],
                        lhsT=vt_sb[kt][:, ct * 128 : (ct + 1) * 128],
                        rhs=attn_sb[kt][:],
                        start=(kt == 0), stop=(kt == 4),
                        skip_group_check=True,
                    )
                nc.vector.tensor_copy(out=osb[ct][:], in_=po[:])
                (nc.sync if ct == 0 else nc.scalar).dma_start(
                    out=outd.ap()[ct * 128 : (ct + 1) * 128, :], in_=osb[ct][:]
                )
    nc.finalize()
    return nc


def _prep_in_maps(key, query, value, Wk, bk, Wq, bq, wf, bf):
    f32, f16 = np.float32, np.float16
    key = np.ascontiguousarray(key, f32).reshape(B, C, NK)
    query = np.ascontiguousarray(query, f32).reshape(B, C, HW, HW)
    value = np.ascontiguousarray(value, f32).reshape(B, C, NK)
    Wk = np.asarray(Wk, f32)
    Wq = np.asarray(Wq, f32)
    wf = np.asarray(wf, f32)
    bk = np.asarray(bk, f32)
    bq = np.asarray(bq, f32)
    bf = np.float32(bf)

    wkt2 = np.concatenate([Wk.T, Wk.T], axis=1)  # (256, 128)
    wqt2 = np.concatenate([Wq.T, Wq.T], axis=1)
    wkq = np.ascontiguousarray(np.concatenate([wkt2, wqt2], axis=1)).astype(f16)

    # linear-term vectors (biases folded into bf_eff)
    u = (A_LIN * wf) @ Wk   # (256,)
    v2 = (A_LIN * wf) @ Wq
    uv = np.zeros((C, NQ + 128), f32)
    uv[:, 0:NQ] = u[:, None]
    uv[:, NQ : NQ + 128] = v2[:, None]
    uv = np.ascontiguousarray(uv).astype(f16)

    # rank-2 bias add into the duplicated k_/q_ psum
    brow = np.zeros((2, NK + NQ + 128), f32)
    brow[0, 0:NK] = 1.0
    brow[1, NK : NK + NQ] = 1.0
    brow[0, NK + NQ :] = np.tile(bk, 2)
    brow[1, NK + NQ :] = np.tile(bq, 2)
    brow = brow.astype(f16)

    cpk = np.zeros((128, R + 4), f32)
    for r in range(R):
        cpk[:64, r] = BB[r] * wf      # + sign: sin(x+pi/4) rows
        cpk[64:, r] = -BB[r] * wf     # - sign: sin(x+3pi/4)=cos(x+pi/4) rows
    cpk[:64, R] = 0.125               # phase, in turns
    cpk[64:, R] = 0.375
    bf_eff = bf + A_LIN * float(wf @ (bk + bq))
    cpk[:, R + 1] = 0.5 * bf_eff

    key16 = key.astype(f16)
    query16 = query.astype(f16)
    common = {"wkq": wkq, "uvin": uv, "biasrow": brow}
    in_maps = []
    for i in range(8):
        b, h = i // 2, i % 2
        qs = np.ascontiguousarray(
            query16[b, :, h * 12 : (h + 1) * 12, :]
        ).reshape(C, NQ)
        vsum = 0.5 * value[b].sum(axis=1)
        valt05 = np.ascontiguousarray(
            np.concatenate([(0.5 * value[b]).T, vsum[None, :]], axis=0)
        ).astype(f16)
        m = {
            "kqin": np.ascontiguousarray(np.concatenate([key16[b], qs], axis=1)),
            "valtin": valt05,
            "cpack": cpk,
        }
        m.update(common)
        in_maps.append(m)
    return in_maps


def run(trace=False, **inputs):
    from concourse.bass_utils import run_bass_kernel_spmd

    inputs.pop("mode", None)
    inputs.pop("chunk", None)
    if "nc" not in _cache:
        _cache["nc"] = _build()
    nc = _cache["nc"]
    in_maps = _prep_in_maps(**inputs)
    res = run_bass_kernel_spmd(nc, in_maps, core_ids=list(range(8)), trace=trace)
    out = np.empty((B, C, HW, HW), np.float32)
    for i in range(8):
        b, h = i // 2, i % 2
        out[b, :, h * 12 : (h + 1) * 12, :] = res.results[i]["out"].reshape(C, 12, HW)
    return out, res


def kernel(**inputs):
    out, _ = run(**inputs)
    return out


# revision 24
# speedup vs baseline: 1.1100x; 1.0514x over previous
"""Additive attention (B=4, C=256, CO=64, H=W=24) on 8 TRN2 NeuronCores.

Sharding: core i handles batch b = i // 2 and Nq-half h = i % 2 (rows
12h..12h+12 of the 24x24 query grid). Each core produces a complete
(256, 288) slice of the output; no collectives are needed.

Algorithm (Fourier-factorized additive attention): the score tensor
  scores[k, q] = sum_c wf_c * tanh(k_c[k] + q_c[q])
is O(Nk*Nq*CO) elementwise work if computed directly (the tanh alone is
~69us/core on the ACT engine). Instead approximate
  tanh(x) ~= a*x + sum_r b_r sin(om_r x)
(free-frequency least-squares fit, weighted by the N(0,2) density of
x = k_c + q_c; R=5 gives weighted-RMS error 9.3e-4) and use
  sin(om(k+q)) = sin(om k)cos(om q) + cos(om k)sin(om q),
which factorizes scores into a rank-(2R*CO + 2) matmul:
  scores = F(k)^T G(q) + a*(Ak[k] + Aq[q]),
with F/G = {sin,cos}(om_r * .) feature maps over the 64 channels. The
O(N^2 C) tanh becomes an O(N^2 * 2R*C) PE matmul plus O(N*C*R)
elementwise sin work - engines: PE ~8us, ACT ~7us, DVE ~6us per core.

Range reduction for sin: a custom DVE op (FRAC_SHIFT_ANT, registered at
runtime) computes f = y - round(y) with y = x*(om/2pi) + phase/2pi via
the fp32 magic-constant rounding trick; ACT then evaluates
sin(2pi * f), arg range exactly [-pi, pi] (the ACT Sin table diverges
beyond ~|3.5| rad). Both quadratures ride one wrap+sin per r via
sin(a+b) = sin(a+pi/4)sin(b+pi/4) - cos(a+pi/4)cos(b+pi/4) (phase
vector 1/8 | 3/8 turns; the minus sign folds into the q-side scale).

sigmoid(s) is computed as 0.5 + 0.5*tanh(0.5 s) (a Silu warmup pins
the one ACT table holding Sin+Tanh+Silu -> no mid-kernel table
reloads); the 0.5 offset rides the value matmul as a 65th
(0.5*rowsum(value) x ones) contraction row, and the 0.5 factor folds
into the host-transposed value tiles. k_/q_ biases enter via a rank-2
matmul into the duplicated k_/q_ psum; the tanh linear term comes from
the raw inputs via host-folded vectors u = Wk^T(a*wf), v2 = Wq^T(a*wf).

Measured: ~29 us exec (neuron-profile; baseline 122 us), rel err
~1.2e-3 (gate 2e-2). Engine budget per core: DVE wraps ~4.5us,
ACT sin+sigmoid ~6us, PE matmuls ~8us, framework preamble/teardown
~10us, input-DMA latency ~4.5us.
"""

import numpy as np

B, C, CO, HW = 4, 256, 64, 24
NK = 576
NQ = 288  # per-core query count (half of 576)
KT_SIZES = [128, 128, 128, 128, 64]

# tanh(x) ~= A_LIN*x + sum_r BB[r]*sin(OM[r]*x); weighted LSQ fit on N(0,2)
# R=4: weighted-RMS 2.4e-3, end-to-end rel err ~1.2e-3 (R=5 alt: 0.18780 /
# [0.589796,1.188114,1.868618,2.723939,3.824876] /
# [0.553768,0.196597,0.080735,0.02663,0.006279] -> 5.3e-4)
A_LIN = 0.18960
OM = [0.595782, 1.259669, 2.109728, 3.210177]
BB = [0.561325, 0.210306, 0.069877, 0.016487]
R = len(OM)
TWO_PI = float(2.0 * np.pi)
MAGIC = 12582912.0  # 3 * 2^22: fp32 round-to-nearest-integer constant

_cache = {}


def _register_frac_op():
    """Register the FRAC_SHIFT_ANT custom DVE op (idempotent):
    out = y - round(y), y = in0*s0 + s1  (all fp32; round via +/-MAGIC).
    """
    import concourse.dve_ops as dve_ops
    from concourse.dve_spec import Spec, Src0, C0, C1, C2, lower
    from concourse.dve_uop import DveOpSpec

    for op in dve_ops.OPS:
        if op.name == "FRAC_SHIFT_ANT":
            return op

    y = Src0 * C0 + C1
    n = (y + C2) - C2
    spec = Spec(
        body=y - n,
        reference=lambda in0, in1, s0, s1, imm2: (
            lambda yy: yy
            - ((yy + np.float32(imm2)).astype(np.float32) - np.float32(imm2))
        )((np.float32(in0) * np.float32(s0) + np.float32(s1)).astype(np.float32)),
    )
    opcode = dve_ops._CUSTOM_DVE_ROW_BASE + len(dve_ops.OPS)
    shas = {}
    for ver in ("v3", "v4"):
        shas[ver] = DveOpSpec(
            name="FRAC_SHIFT_ANT", opcode=opcode, uops=lower(spec, ver=ver),
            rd1_en=False,
        ).sha(ver)
    op = dve_ops.DveOp("FRAC_SHIFT_ANT", spec, subdim=False, uops_sha=shas)
    dve_ops.OPS.append(op)
    dve_ops.CUSTOM_DVE_SPECS[op.name] = op.spec
    dve_ops._SUB_OPCODE_FOR_NAME[op.name] = opcode
    return op


def _build():
    import concourse.bacc as bacc
    import concourse.mybir as mybir
    from concourse.tile import TileContext

    frac_op = _register_frac_op()

    f32 = mybir.dt.float32
    f16 = mybir.dt.float16
    AF = mybir.ActivationFunctionType

    nc = bacc.Bacc("TRN2", target_bir_lowering=False, debug=False, num_devices=8)
    with TileContext(nc) as tc:
        kqin = nc.dram_tensor("kqin", [C, NK + NQ], f16, kind="ExternalInput")
        valtin = nc.dram_tensor("valtin", [NK + 1, C], f16, kind="ExternalInput")
        wkq = nc.dram_tensor("wkq", [C, 256], f16, kind="ExternalInput")
        # uv: cols 0:NQ = ubc (u[cin] bcast over q), NQ:NQ+128 = v2bc
        uvin = nc.dram_tensor("uvin", [C, NQ + 128], f16, kind="ExternalInput")
        # biasrow: rank-2 bias add into pkq. cols 0:864 = rhs ([ones|0;0|ones]
        # masked), 864:992 = lhsT ([bk-dup; bq-dup])
        biasrow = nc.dram_tensor("biasrow", [2, NK + NQ + 128], f16, kind="ExternalInput")
        # cpack f32: cols 0:R = qscale (+-b_r wf dup), R = phase vec (1/8,
        # 3/8 turns), R+1 = bfv, R+2:R+4 = vsum05 per ct
        cpack = nc.dram_tensor("cpack", [128, R + 4], f32, kind="ExternalInput")
        outd = nc.dram_tensor("out", [C, NQ], f16, kind="ExternalOutput")

        with (
            tc.tile_pool(name="consts", bufs=1) as consts,
            tc.tile_pool(name="inp", bufs=1) as inp,
            tc.tile_pool(name="work", bufs=1) as work,
            tc.tile_pool(name="wp", bufs=2) as wp,
            tc.tile_pool(name="fp", bufs=3) as fp,
            tc.tile_pool(name="gp", bufs=2) as gp,
            tc.tile_pool(name="psum", bufs=1, space="PSUM") as psum,
        ):
            kq_sb = [inp.tile([128, NK + NQ], f16, tag=f"kq{t}", name=f"kq{t}") for t in range(2)]
            vt_sb = [
                inp.tile([KT_SIZES[kt] + (kt == 4), C], f16, tag=f"vt{kt}", name=f"vt{kt}")
                for kt in range(5)
            ]
            wkq_sb = [consts.tile([128, 256], f16, tag=f"wkq{t}", name=f"wkq{t}") for t in range(2)]
            uv_sb = [consts.tile([128, NQ + 128], f16, tag=f"uv{t}", name=f"uv{t}") for t in range(2)]
            br_sb = consts.tile([2, NK + NQ + 128], f16, tag="br")
            cp_sb = consts.tile([128, R + 4], f32, tag="cp")
            scr = consts.tile([128, 1], f32, tag="scr")
            warm = consts.tile([128, 512], f16, tag="warm")

            # DMA issue order matters per queue (SP / ACT / Pool streams).
            nc.sync.dma_start(out=br_sb[:], in_=biasrow.ap())
            nc.scalar.dma_start(out=wkq_sb[0][:], in_=wkq.ap()[0:128, :])
            nc.gpsimd.dma_start(out=kq_sb[1][:], in_=kqin.ap()[128:256, :])
            nc.sync.dma_start(out=kq_sb[0][:], in_=kqin.ap()[0:128, :])
            nc.scalar.dma_start(out=wkq_sb[1][:], in_=wkq.ap()[128:256, :])
            nc.sync.dma_start(out=cp_sb[:], in_=cpack.ap())
            nc.sync.dma_start(out=uv_sb[0][:], in_=uvin.ap()[0:128, :])
            nc.scalar.dma_start(out=uv_sb[1][:], in_=uvin.ap()[128:256, :])
            # warmup: Silu pins the silu_and_others ACT table (sin+tanh+silu
            # in one table -> no mid-kernel ACT_TABLE_LOADs)
            nc.vector.memset(scr[:], 0.0)
            nc.scalar.activation(scr[:], scr[:], AF.Silu)
            nc.vector.memset(warm[:], 0.0)
            for kt in range(5):
                nc.gpsimd.dma_start(
                    out=vt_sb[kt][:],
                    in_=valtin.ap()[kt * 128 : kt * 128 + KT_SIZES[kt] + (kt == 4), :],
                )

            # attn4/vt4 carry a 65th row (ones / 0.5*vsum) so the value
            # matmul adds the sigmoid 0.5-offset correction for free
            attn_sb = [
                work.tile([KT_SIZES[kt] + (kt == 4), NQ], f16, tag=f"attn{kt}", name=f"attn{kt}")
                for kt in range(5)
            ]
            nc.vector.memset(attn_sb[4][64:65, :], 1.0)
            osb = [work.tile([128, NQ], f16, tag=f"osb{t}", name=f"osb{t}") for t in range(2)]

            pkq_t = psum.tile([128, NK + NQ], f32, tag="pkq")
            scores = [
                psum.tile([KT_SIZES[kt], NQ], f32, tag=f"sc{kt}", name=f"sc{kt}")
                for kt in range(5)
            ]

            # PE p-state warmup in the DMA shadow (throwaway group in pkq_t)
            for i in range(8):
                nc.tensor.matmul(
                    out=pkq_t[:, 0:512], lhsT=warm[:, 0:128], rhs=warm[:],
                    start=(i == 0), stop=(i == 7),
                )

            # k_/q_ = [W|W]^T @ (key|qry) + rank-2 bias add -> dup rows psum
            for c0, c1 in ((0, 512), (512, NK), (NK, NK + NQ)):
                col = 0 if c1 <= NK else 128
                nc.tensor.matmul(
                    out=pkq_t[:, c0:c1],
                    lhsT=br_sb[:, NK + NQ : NK + NQ + 128],
                    rhs=br_sb[:, c0:c1],
                    start=True, stop=False, skip_group_check=True,
                )
                for ct in range(2):
                    nc.tensor.matmul(
                        out=pkq_t[:, c0:c1],
                        lhsT=wkq_sb[ct][:, col : col + 128],
                        rhs=kq_sb[ct][:, c0:c1],
                        start=False, stop=(ct == 1), skip_group_check=True,
                    )

            # linear term, from the raw inputs (PE-ready at DMA land):
            # scores[k,q] += u^T key (bcast q) + v2^T qry (bcast k)
            for kt in range(5):
                ks = slice(kt * 128, kt * 128 + KT_SIZES[kt])
                for ct in range(2):
                    nc.tensor.matmul(
                        out=scores[kt][:], lhsT=kq_sb[ct][:, ks],
                        rhs=uv_sb[ct][:, 0:NQ],
                        start=(ct == 0), stop=False, skip_group_check=True,
                    )
                for ct in range(2):
                    nc.tensor.matmul(
                        out=scores[kt][:],
                        lhsT=uv_sb[ct][:, NQ : NQ + KT_SIZES[kt]],
                        rhs=kq_sb[ct][:, NK : NK + NQ],
                        start=False, stop=False, skip_group_check=True,
                    )

            # Fourier features: one FRAC_SHIFT per r over [k|q] (the pi/4
            # phase trick makes both sides share one phase vector; the sign
            # lands in qscale), one Sin ACT per r, one q-scale per r.
            wr_t = [None] * R

            def emit_wrap(r):
                wr = wp.tile([128, NK + NQ], f32, tag="wr")
                nc.vector._custom_dve(
                    frac_op, out=wr[:], in0=pkq_t[:],
                    s0=float(OM[r] / TWO_PI), s1=cp_sb[:, R : R + 1], imm2=MAGIC,
                )
                wr_t[r] = wr

            emit_wrap(0)
            fr_t = [None] * R

            def emit_tail(r):
                # q-feature scale + the 5 score matmuls for round r
                fr = fr_t[r]
                gr = gp.tile([128, NQ], f16, tag="gr")
                if r == 0:
                    nc.scalar.activation(
                        gr[:], fr[:, NK : NK + NQ], AF.Identity,
                        scale=cp_sb[:, r : r + 1],
                    )
                else:
                    nc.vector.tensor_scalar_mul(
                        out=gr[:], in0=fr[:, NK : NK + NQ],
                        scalar1=cp_sb[:, r : r + 1],
                    )
                for kt in range(5):
                    ks = slice(kt * 128, kt * 128 + KT_SIZES[kt])
                    nc.tensor.matmul(
                        out=scores[kt][:], lhsT=fr[:, ks], rhs=gr[:],
                        start=False, stop=(r == R - 1),
                        skip_group_check=True,
                    )

            for r in range(R):
                fr = fp.tile([128, NK + NQ], f16, tag="fr")
                nc.scalar.activation(fr[:], wr_t[r][:], AF.Sin, scale=TWO_PI)
                fr_t[r] = fr
                if r + 1 < R:
                    emit_wrap(r + 1)
                if r >= 1:
                    emit_tail(r - 1)
            emit_tail(R - 1)

            # attn_t = tanh(0.5*scores + bfv); sigmoid = 0.5 + 0.5*attn_t
            for kt in range(5):
                nc.scalar.activation(
                    attn_sb[kt][: KT_SIZES[kt], :], scores[kt][:], AF.Tanh,
                    scale=0.5, bias=cp_sb[: KT_SIZES[kt], R + 1 : R + 2],
                )

            # out = (0.5*value | 0.5*vsum) @ (attn_t | ones): the 65th row of
            # the kt=4 pair adds the 0.5*vsum offset; output DMAs straight
            # from psum (reusing scores[ct]'s bank)
            for ct in range(2):
                po = scores[ct]
                for kt in range(5):
                    nc.tensor.matmul(
                        out=po[:],
                        lhsT=vt_sb[kt][:, ct * 128 : (ct + 1) * 128],
                        rhs=attn_sb[kt][:],
                        start=(kt == 0), stop=(kt == 4),
                        skip_group_check=True,
                    )
                nc.vector.tensor_copy(out=osb[ct][:], in_=po[:])
                (nc.sync if ct == 0 else nc.scalar).dma_start(
                    out=outd.ap()[ct * 128 : (ct + 1) * 128, :], in_=osb[ct][:]
                )
    nc.finalize()
    return nc


def _prep_in_maps(key, query, value, Wk, bk, Wq, bq, wf, bf):
    f32, f16 = np.float32, np.float16
    key = np.ascontiguousarray(key, f32).reshape(B, C, NK)
    query = np.ascontiguousarray(query, f32).reshape(B, C, HW, HW)
    value = np.ascontiguousarray(value, f32).reshape(B, C, NK)
    Wk = np.asarray(Wk, f32)
    Wq = np.asarray(Wq, f32)
    wf = np.asarray(wf, f32)
    bk = np.asarray(bk, f32)
    bq = np.asarray(bq, f32)
    bf = np.float32(bf)

    wkt2 = np.concatenate([Wk.T, Wk.T], axis=1)  # (256, 128)
    wqt2 = np.concatenate([Wq.T, Wq.T], axis=1)
    wkq = np.ascontiguousarray(np.concatenate([wkt2, wqt2], axis=1)).astype(f16)

    # linear-term vectors (biases folded into bf_eff)
    u = (A_LIN * wf) @ Wk   # (256,)
    v2 = (A_LIN * wf) @ Wq
    uv = np.zeros((C, NQ + 128), f32)
    uv[:, 0:NQ] = u[:, None]
    uv[:, NQ : NQ + 128] = v2[:, None]
    uv = np.ascontiguousarray(uv).astype(f16)

    # rank-2 bias add into the duplicated k_/q_ psum
    brow = np.zeros((2, NK + NQ + 128), f32)
    brow[0, 0:NK] = 1.0
    brow[1, NK : NK + NQ] = 1.0
    brow[0, NK + NQ :] = np.tile(bk, 2)
    brow[1, NK + NQ :] = np.tile(bq, 2)
    brow = brow.astype(f16)

    cpk = np.zeros((128, R + 4), f32)
    for r in range(R):
        cpk[:64, r] = BB[r] * wf      # + sign: sin(x+pi/4) rows
        cpk[64:, r] = -BB[r] * wf     # - sign: sin(x+3pi/4)=cos(x+pi/4) rows
    cpk[:64, R] = 0.125               # phase, in turns
    cpk[64:, R] = 0.375
    bf_eff = bf + A_LIN * float(wf @ (bk + bq))
    cpk[:, R + 1] = 0.5 * bf_eff

    key16 = key.astype(f16)
    query16 = query.astype(f16)
    common = {"wkq": wkq, "uvin": uv, "biasrow": brow}
    in_maps = []
    for i in range(8):
        b, h = i // 2, i % 2
        qs = np.ascontiguousarray(
            query16[b, :, h * 12 : (h + 1) * 12, :]
        ).reshape(C, NQ)
        vsum = 0.5 * value[b].sum(axis=1)
        valt05 = np.ascontiguousarray(
            np.concatenate([(0.5 * value[b]).T, vsum[None, :]], axis=0)
        ).astype(f16)
        m = {
            "kqin": np.ascontiguousarray(np.concatenate([key16[b], qs], axis=1)),
            "valtin": valt05,
            "cpack": cpk,
        }
        m.update(common)
        in_maps.append(m)
    return in_maps


def run(trace=False, **inputs):
    from concourse.bass_utils import run_bass_kernel_spmd

    inputs.pop("mode", None)
    inputs.pop("chunk", None)
    if "nc" not in _cache:
        _cache["nc"] = _build()
    nc = _cache["nc"]
    in_maps = _prep_in_maps(**inputs)
    res = run_bass_kernel_spmd(nc, in_maps, core_ids=list(range(8)), trace=trace)
    out = np.empty((B, C, HW, HW), np.float32)
    for i in range(8):
        b, h = i // 2, i % 2
        out[b, :, h * 12 : (h + 1) * 12, :] = (
            res.results[i]["out"].astype(np.float32).reshape(C, 12, HW)
        )
    return out, res


def kernel(**inputs):
    out, _ = run(**inputs)
    return out


# revision 26
# speedup vs baseline: 1.1816x; 1.0645x over previous
"""Additive attention (B=4, C=256, CO=64, H=W=24) on 8 TRN2 NeuronCores.

Sharding: core i handles batch b = i // 2 and Nq-half h = i % 2 (rows
12h..12h+12 of the 24x24 query grid). Each core produces a complete
(256, 288) slice of the output; no collectives are needed.

Algorithm (Fourier-factorized additive attention): the score tensor
  scores[k, q] = sum_c wf_c * tanh(k_c[k] + q_c[q])
is O(Nk*Nq*CO) elementwise work if computed directly (the tanh alone is
~69us/core on the ACT engine). Instead approximate
  tanh(x) ~= a*x + sum_r b_r sin(om_r x)
(free-frequency least-squares fit, weighted by the N(0,2) density of
x = k_c + q_c; R=3 gives weighted-RMS error 6.1e-3) and use
  sin(om(k+q)) = sin(om k)cos(om q) + cos(om k)sin(om q),
which factorizes scores into a rank-(2R*CO + 2) matmul:
  scores = F(k)^T G(q) + a*(Ak[k] + Aq[q]),
with F/G = {sin,cos}(om_r * .) feature maps over the 64 channels. The
O(N^2 C) tanh becomes an O(N^2 * 2R*C) PE matmul plus O(N*C*R)
elementwise sin work - engines: PE ~8us, ACT ~7us, DVE ~6us per core.

Range reduction for sin: a custom DVE op (FRAC_SHIFT_ANT, registered at
runtime) computes f = y - round(y) with y = x*(om/2pi) + phase/2pi via
the fp32 magic-constant rounding trick; ACT then evaluates
sin(2pi * f), arg range exactly [-pi, pi] (the ACT Sin table diverges
beyond ~|3.5| rad). Both quadratures ride one wrap+sin per r via
sin(a+b) = sin(a+pi/4)sin(b+pi/4) - cos(a+pi/4)cos(b+pi/4) (phase
vector 1/8 | 3/8 turns; the minus sign folds into the q-side scale).

sigmoid(s) is computed as 0.5 + 0.5*tanh(0.5 s) (a Silu warmup pins
the one ACT table holding Sin+Tanh+Silu -> no mid-kernel table
reloads); the 0.5 offset rides the value matmul as a 65th
(0.5*rowsum(value) x ones) contraction row, and the 0.5 factor folds
into the host-transposed value tiles. k_/q_ biases enter via a rank-2
matmul into the duplicated k_/q_ psum; the tanh linear term comes from
the raw inputs via host-folded vectors u = Wk^T(a*wf), v2 = Wq^T(a*wf).

Measured: ~27 us exec (neuron-profile; baseline 122 us), rel err
2.9e-3 (gate 2e-2). Engine budget per core: DVE wraps ~3.4us,
ACT sin+sigmoid ~5us, PE matmuls ~7us, framework preamble/teardown
~10us, input-DMA latency ~4.5us.
"""

import numpy as np

B, C, CO, HW = 4, 256, 64, 24
NK = 576
NQ = 288  # per-core query count (half of 576)
KT_SIZES = [128, 128, 128, 128, 64]

# tanh(x) ~= A_LIN*x + sum_r BB[r]*sin(OM[r]*x); free-frequency weighted LSQ
# fit under the N(0,2) density of x = k_c + q_c.
# R=3: weighted-RMS 6.1e-3 -> end-to-end rel err 2.9e-3 (gate 2e-2).
# Alternates if more margin is ever needed:
#  R=4: 0.18960 / [0.595782,1.259669,2.109728,3.210177]
#       / [0.561325,0.210306,0.069877,0.016487]        -> 1.2e-3
#  R=5: 0.18780 / [0.589796,1.188114,1.868618,2.723939,3.824876]
#       / [0.553768,0.196597,0.080735,0.02663,0.006279] -> 5.3e-4
A_LIN = 0.18377
OM = [0.645559, 1.508624, 2.613956]
BB = [0.629316, 0.182934, 0.042099]
R = len(OM)
TWO_PI = float(2.0 * np.pi)
MAGIC = 12582912.0  # 3 * 2^22: fp32 round-to-nearest-integer constant

_cache = {}


def _register_frac_op():
    """Register the FRAC_SHIFT_ANT custom DVE op (idempotent):
    out = y - round(y), y = in0*s0 + s1  (all fp32; round via +/-MAGIC).
    """
    import concourse.dve_ops as dve_ops
    from concourse.dve_spec import Spec, Src0, C0, C1, C2, lower
    from concourse.dve_uop import DveOpSpec

    for op in dve_ops.OPS:
        if op.name == "FRAC_SHIFT_ANT":
            return op

    y = Src0 * C0 + C1
    n = (y + C2) - C2
    spec = Spec(
        body=y - n,
        reference=lambda in0, in1, s0, s1, imm2: (
            lambda yy: yy
            - ((yy + np.float32(imm2)).astype(np.float32) - np.float32(imm2))
        )((np.float32(in0) * np.float32(s0) + np.float32(s1)).astype(np.float32)),
    )
    opcode = dve_ops._CUSTOM_DVE_ROW_BASE + len(dve_ops.OPS)
    shas = {}
    for ver in ("v3", "v4"):
        shas[ver] = DveOpSpec(
            name="FRAC_SHIFT_ANT", opcode=opcode, uops=lower(spec, ver=ver),
            rd1_en=False,
        ).sha(ver)
    op = dve_ops.DveOp("FRAC_SHIFT_ANT", spec, subdim=False, uops_sha=shas)
    dve_ops.OPS.append(op)
    dve_ops.CUSTOM_DVE_SPECS[op.name] = op.spec
    dve_ops._SUB_OPCODE_FOR_NAME[op.name] = opcode
    return op


def _build():
    import concourse.bacc as bacc
    import concourse.mybir as mybir
    from concourse.tile import TileContext

    frac_op = _register_frac_op()

    f32 = mybir.dt.float32
    f16 = mybir.dt.float16
    AF = mybir.ActivationFunctionType

    nc = bacc.Bacc("TRN2", target_bir_lowering=False, debug=False, num_devices=8)
    with TileContext(nc) as tc:
        kqin = nc.dram_tensor("kqin", [C, NK + NQ], f16, kind="ExternalInput")
        valtin = nc.dram_tensor("valtin", [NK + 1, C], f16, kind="ExternalInput")
        wkq = nc.dram_tensor("wkq", [C, 256], f16, kind="ExternalInput")
        # uv: cols 0:NQ = ubc (u[cin] bcast over q), NQ:NQ+128 = v2bc
        uvin = nc.dram_tensor("uvin", [C, NQ + 128], f16, kind="ExternalInput")
        # biasrow: rank-2 bias add into pkq. cols 0:864 = rhs ([ones|0;0|ones]
        # masked), 864:992 = lhsT ([bk-dup; bq-dup])
        biasrow = nc.dram_tensor("biasrow", [2, NK + NQ + 128], f16, kind="ExternalInput")
        # cpack f32: cols 0:R = qscale (+-b_r wf dup), R = phase vec (1/8,
        # 3/8 turns), R+1 = bfv, R+2:R+4 = vsum05 per ct
        cpack = nc.dram_tensor("cpack", [128, R + 4], f32, kind="ExternalInput")
        outd = nc.dram_tensor("out", [C, NQ], f16, kind="ExternalOutput")

        with (
            tc.tile_pool(name="consts", bufs=1) as consts,
            tc.tile_pool(name="inp", bufs=1) as inp,
            tc.tile_pool(name="work", bufs=1) as work,
            tc.tile_pool(name="wp", bufs=2) as wp,
            tc.tile_pool(name="fp", bufs=3) as fp,
            tc.tile_pool(name="gp", bufs=2) as gp,
            tc.tile_pool(name="psum", bufs=1, space="PSUM") as psum,
        ):
            kq_sb = [inp.tile([128, NK + NQ], f16, tag=f"kq{t}", name=f"kq{t}") for t in range(2)]
            vt_sb = [
                inp.tile([KT_SIZES[kt] + (kt == 4), C], f16, tag=f"vt{kt}", name=f"vt{kt}")
                for kt in range(5)
            ]
            wkq_sb = [consts.tile([128, 256], f16, tag=f"wkq{t}", name=f"wkq{t}") for t in range(2)]
            uv_sb = [consts.tile([128, NQ + 128], f16, tag=f"uv{t}", name=f"uv{t}") for t in range(2)]
            br_sb = consts.tile([2, NK + NQ + 128], f16, tag="br")
            cp_sb = consts.tile([128, R + 4], f32, tag="cp")
            scr = consts.tile([128, 1], f32, tag="scr")
            warm = consts.tile([128, 512], f16, tag="warm")

            # DMA issue order matters per queue (SP / ACT / Pool streams).
            nc.sync.dma_start(out=br_sb[:], in_=biasrow.ap())
            nc.scalar.dma_start(out=wkq_sb[0][:], in_=wkq.ap()[0:128, :])
            nc.gpsimd.dma_start(out=kq_sb[1][:], in_=kqin.ap()[128:256, :])
            nc.sync.dma_start(out=kq_sb[0][:], in_=kqin.ap()[0:128, :])
            nc.scalar.dma_start(out=wkq_sb[1][:], in_=wkq.ap()[128:256, :])
            nc.sync.dma_start(out=cp_sb[:], in_=cpack.ap())
            nc.sync.dma_start(out=uv_sb[0][:], in_=uvin.ap()[0:128, :])
            nc.scalar.dma_start(out=uv_sb[1][:], in_=uvin.ap()[128:256, :])
            # warmup: Silu pins the silu_and_others ACT table (sin+tanh+silu
            # in one table -> no mid-kernel ACT_TABLE_LOADs)
            nc.vector.memset(scr[:], 0.0)
            nc.scalar.activation(scr[:], scr[:], AF.Silu)
            nc.vector.memset(warm[:], 0.0)
            for kt in range(5):
                nc.gpsimd.dma_start(
                    out=vt_sb[kt][:],
                    in_=valtin.ap()[kt * 128 : kt * 128 + KT_SIZES[kt] + (kt == 4), :],
                )

            # attn4/vt4 carry a 65th row (ones / 0.5*vsum) so the value
            # matmul adds the sigmoid 0.5-offset correction for free
            attn_sb = [
                work.tile([KT_SIZES[kt] + (kt == 4), NQ], f16, tag=f"attn{kt}", name=f"attn{kt}")
                for kt in range(5)
            ]
            nc.vector.memset(attn_sb[4][64:65, :], 1.0)
            osb = [work.tile([128, NQ], f16, tag=f"osb{t}", name=f"osb{t}") for t in range(2)]

            pkq_t = psum.tile([128, NK + NQ], f32, tag="pkq")
            scores = [
                psum.tile([KT_SIZES[kt], NQ], f32, tag=f"sc{kt}", name=f"sc{kt}")
                for kt in range(5)
            ]

            # PE p-state warmup in the DMA shadow (throwaway group in pkq_t)
            for i in range(8):
                nc.tensor.matmul(
                    out=pkq_t[:, 0:512], lhsT=warm[:, 0:128], rhs=warm[:],
                    start=(i == 0), stop=(i == 7),
                )

            # k_/q_ = [W|W]^T @ (key|qry) + rank-2 bias add -> dup rows psum
            for c0, c1 in ((0, 512), (512, NK), (NK, NK + NQ)):
                col = 0 if c1 <= NK else 128
                nc.tensor.matmul(
                    out=pkq_t[:, c0:c1],
                    lhsT=br_sb[:, NK + NQ : NK + NQ + 128],
                    rhs=br_sb[:, c0:c1],
                    start=True, stop=False, skip_group_check=True,
                )
                for ct in range(2):
                    nc.tensor.matmul(
                        out=pkq_t[:, c0:c1],
                        lhsT=wkq_sb[ct][:, col : col + 128],
                        rhs=kq_sb[ct][:, c0:c1],
                        start=False, stop=(ct == 1), skip_group_check=True,
                    )

            # linear term, from the raw inputs (PE-ready at DMA land):
            # scores[k,q] += u^T key (bcast q) + v2^T qry (bcast k)
            for kt in range(5):
                ks = slice(kt * 128, kt * 128 + KT_SIZES[kt])
                for ct in range(2):
                    nc.tensor.matmul(
                        out=scores[kt][:], lhsT=kq_sb[ct][:, ks],
                        rhs=uv_sb[ct][:, 0:NQ],
                        start=(ct == 0), stop=False, skip_group_check=True,
                    )
                for ct in range(2):
                    nc.tensor.matmul(
                        out=scores[kt][:],
                        lhsT=uv_sb[ct][:, NQ : NQ + KT_SIZES[kt]],
                        rhs=kq_sb[ct][:, NK : NK + NQ],
                        start=False, stop=False, skip_group_check=True,
                    )

            # Fourier features: one FRAC_SHIFT per r over [k|q] (the pi/4
            # phase trick makes both sides share one phase vector; the sign
            # lands in qscale), one Sin ACT per r, one q-scale per r.
            wr_t = [None] * R

            def emit_wrap(r):
                wr = wp.tile([128, NK + NQ], f32, tag="wr")
                nc.vector._custom_dve(
                    frac_op, out=wr[:], in0=pkq_t[:],
                    s0=float(OM[r] / TWO_PI), s1=cp_sb[:, R : R + 1], imm2=MAGIC,
                )
                wr_t[r] = wr

            emit_wrap(0)
            fr_t = [None] * R

            def emit_tail(r):
                # q-feature scale + the 5 score matmuls for round r
                fr = fr_t[r]
                gr = gp.tile([128, NQ], f16, tag="gr")
                if r == 0:
                    nc.scalar.activation(
                        gr[:], fr[:, NK : NK + NQ], AF.Identity,
                        scale=cp_sb[:, r : r + 1],
                    )
                else:
                    nc.vector.tensor_scalar_mul(
                        out=gr[:], in0=fr[:, NK : NK + NQ],
                        scalar1=cp_sb[:, r : r + 1],
                    )
                for kt in range(5):
                    ks = slice(kt * 128, kt * 128 + KT_SIZES[kt])
                    nc.tensor.matmul(
                        out=scores[kt][:], lhsT=fr[:, ks], rhs=gr[:],
                        start=False, stop=(r == R - 1),
                        skip_group_check=True,
                    )

            for r in range(R):
                fr = fp.tile([128, NK + NQ], f16, tag="fr")
                nc.scalar.activation(fr[:], wr_t[r][:], AF.Sin, scale=TWO_PI)
                fr_t[r] = fr
                if r + 1 < R:
                    emit_wrap(r + 1)
                if r >= 1:
                    emit_tail(r - 1)
            emit_tail(R - 1)

            # attn_t = tanh(0.5*scores + bfv); sigmoid = 0.5 + 0.5*attn_t
            for kt in range(5):
                nc.scalar.activation(
                    attn_sb[kt][: KT_SIZES[kt], :], scores[kt][:], AF.Tanh,
                    scale=0.5, bias=cp_sb[: KT_SIZES[kt], R + 1 : R + 2],
                )

            # out = (0.5*value | 0.5*vsum) @ (attn_t | ones): the 65th row of
            # the kt=4 pair adds the 0.5*vsum offset; output DMAs straight
            # from psum (reusing scores[ct]'s bank)
            for ct in range(2):
                po = scores[ct]
                for kt in range(5):
                    nc.tensor.matmul(
                        out=po[:],
                        lhsT=vt_sb[kt][:, ct * 128 : (ct + 1) * 128],
                        rhs=attn_sb[kt][:],
                        start=(kt == 0), stop=(kt == 4),
                        skip_group_check=True,
                    )
                nc.vector.tensor_copy(out=osb[ct][:], in_=po[:])
                (nc.sync if ct == 0 else nc.scalar).dma_start(
                    out=outd.ap()[ct * 128 : (ct + 1) * 128, :], in_=osb[ct][:]
                )
    nc.finalize()
    return nc


def _prep_in_maps(key, query, value, Wk, bk, Wq, bq, wf, bf):
    f32, f16 = np.float32, np.float16
    key = np.ascontiguousarray(key, f32).reshape(B, C, NK)
    query = np.ascontiguousarray(query, f32).reshape(B, C, HW, HW)
    value = np.ascontiguousarray(value, f32).reshape(B, C, NK)
    Wk = np.asarray(Wk, f32)
    Wq = np.asarray(Wq, f32)
    wf = np.asarray(wf, f32)
    bk = np.asarray(bk, f32)
    bq = np.asarray(bq, f32)
    bf = np.float32(bf)

    wkt2 = np.concatenate([Wk.T, Wk.T], axis=1)  # (256, 128)
    wqt2 = np.concatenate([Wq.T, Wq.T], axis=1)
    wkq = np.ascontiguousarray(np.concatenate([wkt2, wqt2], axis=1)).astype(f16)

    # linear-term vectors (biases folded into bf_eff)
    u = (A_LIN * wf) @ Wk   # (256,)
    v2 = (A_LIN * wf) @ Wq
    uv = np.zeros((C, NQ + 128), f32)
    uv[:, 0:NQ] = u[:, None]
    uv[:, NQ : NQ + 128] = v2[:, None]
    uv = np.ascontiguousarray(uv).astype(f16)

    # rank-2 bias add into the duplicated k_/q_ psum
    brow = np.zeros((2, NK + NQ + 128), f32)
    brow[0, 0:NK] = 1.0
    brow[1, NK : NK + NQ] = 1.0
    brow[0, NK + NQ :] = np.tile(bk, 2)
    brow[1, NK + NQ :] = np.tile(bq, 2)
    brow = brow.astype(f16)

    cpk = np.zeros((128, R + 4), f32)
    for r in range(R):
        cpk[:64, r] = BB[r] * wf      # + sign: sin(x+pi/4) rows
        cpk[64:, r] = -BB[r] * wf     # - sign: sin(x+3pi/4)=cos(x+pi/4) rows
    cpk[:64, R] = 0.125               # phase, in turns
    cpk[64:, R] = 0.375
    bf_eff = bf + A_LIN * float(wf @ (bk + bq))
    cpk[:, R + 1] = 0.5 * bf_eff

    key16 = key.astype(f16)
    query16 = query.astype(f16)
    common = {"wkq": wkq, "uvin": uv, "biasrow": brow}
    in_maps = []
    for i in range(8):
        b, h = i // 2, i % 2
        qs = np.ascontiguousarray(
            query16[b, :, h * 12 : (h + 1) * 12, :]
        ).reshape(C, NQ)
        vsum = 0.5 * value[b].sum(axis=1)
        valt05 = np.ascontiguousarray(
            np.concatenate([(0.5 * value[b]).T, vsum[None, :]], axis=0)
        ).astype(f16)
        m = {
            "kqin": np.ascontiguousarray(np.concatenate([key16[b], qs], axis=1)),
            "valtin": valt05,
            "cpack": cpk,
        }
        m.update(common)
        in_maps.append(m)
    return in_maps


def run(trace=False, **inputs):
    from concourse.bass_utils import run_bass_kernel_spmd

    inputs.pop("mode", None)
    inputs.pop("chunk", None)
    if "nc" not in _cache:
        _cache["nc"] = _build()
    nc = _cache["nc"]
    in_maps = _prep_in_maps(**inputs)
    res = run_bass_kernel_spmd(nc, in_maps, core_ids=list(range(8)), trace=trace)
    out = np.empty((B, C, HW, HW), np.float32)
    for i in range(8):
        b, h = i // 2, i % 2
        out[b, :, h * 12 : (h + 1) * 12, :] = (
            res.results[i]["out"].astype(np.float32).reshape(C, 12, HW)
        )
    return out, res


def kernel(**inputs):
    out, _ = run(**inputs)
    return out


# revision 28
# speedup vs baseline: 1.1886x; 1.0059x over previous
"""Additive attention (B=4, C=256, CO=64, H=W=24) on 8 TRN2 NeuronCores.

Sharding: core i handles batch b = i // 2 and Nq-half h = i % 2 (rows
12h..12h+12 of the 24x24 query grid). Each core produces a complete
(256, 288) slice of the output; no collectives are needed.

Algorithm (Fourier-factorized additive attention): the score tensor
  scores[k, q] = sum_c wf_c * tanh(k_c[k] + q_c[q])
is O(Nk*Nq*CO) elementwise work if computed directly (the tanh alone is
~69us/core on the ACT engine). Instead approximate
  tanh(x) ~= a*x + sum_r b_r sin(om_r x)
(free-frequency least-squares fit, weighted by the N(0,2) density of
x = k_c + q_c; R=3 gives weighted-RMS error 6.1e-3) and use
  sin(om(k+q)) = sin(om k)cos(om q) + cos(om k)sin(om q),
which factorizes scores into a rank-(2R*CO + 2) matmul:
  scores = F(k)^T G(q) + a*(Ak[k] + Aq[q]),
with F/G = {sin,cos}(om_r * .) feature maps over the 64 channels. The
O(N^2 C) tanh becomes an O(N^2 * 2R*C) PE matmul plus O(N*C*R)
elementwise sin work - engines: PE ~8us, ACT ~7us, DVE ~6us per core.

Range reduction for sin: a custom DVE op (FRAC_SHIFT_ANT, registered at
runtime) computes f = y - round(y) with y = x*(om/2pi) + phase/2pi via
the fp32 magic-constant rounding trick; ACT then evaluates
sin(2pi * f), arg range exactly [-pi, pi] (the ACT Sin table diverges
beyond ~|3.5| rad). Both quadratures ride one wrap+sin per r via
sin(a+b) = sin(a+pi/4)sin(b+pi/4) - cos(a+pi/4)cos(b+pi/4) (phase
vector 1/8 | 3/8 turns; the minus sign folds into the q-side scale).

sigmoid(s) is computed as 0.5 + 0.5*tanh(0.5 s) (a Silu warmup pins
the one ACT table holding Sin+Tanh+Silu -> no mid-kernel table
reloads); the 0.5 offset rides the value matmul as a 65th
(0.5*rowsum(value) x ones) contraction row, and the 0.5 factor folds
into the host-transposed value tiles. k_/q_ biases enter via a rank-2
matmul into the duplicated k_/q_ psum; the tanh linear term comes from
the raw inputs via host-folded vectors u = Wk^T(a*wf), v2 = Wq^T(a*wf).

Measured: ~27 us exec (neuron-profile; baseline 122 us), rel err
2.9e-3 (gate 2e-2). Engine budget per core: DVE wraps ~3.4us,
ACT sin+sigmoid ~5us, PE matmuls ~7us, framework preamble/teardown
~10us, input-DMA latency ~4.5us.
"""

import numpy as np

B, C, CO, HW = 4, 256, 64, 24
NK = 576
NQ = 288  # per-core query count (half of 576)
KT_SIZES = [128, 128, 128, 128, 64]

# tanh(x) ~= A_LIN*x + sum_r BB[r]*sin(OM[r]*x); free-frequency weighted LSQ
# fit under the N(0,2) density of x = k_c + q_c.
# R=3: weighted-RMS 6.1e-3 -> end-to-end rel err 2.9e-3 (gate 2e-2).
# Alternates if more margin is ever needed:
#  R=4: 0.18960 / [0.595782,1.259669,2.109728,3.210177]
#       / [0.561325,0.210306,0.069877,0.016487]        -> 1.2e-3
#  R=5: 0.18780 / [0.589796,1.188114,1.868618,2.723939,3.824876]
#       / [0.553768,0.196597,0.080735,0.02663,0.006279] -> 5.3e-4
A_LIN = 0.18377
OM = [0.645559, 1.508624, 2.613956]
BB = [0.629316, 0.182934, 0.042099]
R = len(OM)
TWO_PI = float(2.0 * np.pi)
MAGIC = 12582912.0  # 3 * 2^22: fp32 round-to-nearest-integer constant

_cache = {}


def _register_frac_op():
    """Register the FRAC_SHIFT_ANT custom DVE op (idempotent):
    out = y - round(y), y = in0*s0 + s1  (all fp32; round via +/-MAGIC).
    """
    import concourse.dve_ops as dve_ops
    from concourse.dve_spec import Spec, Src0, C0, C1, C2, lower
    from concourse.dve_uop import DveOpSpec

    for op in dve_ops.OPS:
        if op.name == "FRAC_SHIFT_ANT":
            return op

    y = Src0 * C0 + C1
    n = (y + C2) - C2
    spec = Spec(
        body=y - n,
        reference=lambda in0, in1, s0, s1, imm2: (
            lambda yy: yy
            - ((yy + np.float32(imm2)).astype(np.float32) - np.float32(imm2))
        )((np.float32(in0) * np.float32(s0) + np.float32(s1)).astype(np.float32)),
    )
    opcode = dve_ops._CUSTOM_DVE_ROW_BASE + len(dve_ops.OPS)
    shas = {}
    for ver in ("v3", "v4"):
        shas[ver] = DveOpSpec(
            name="FRAC_SHIFT_ANT", opcode=opcode, uops=lower(spec, ver=ver),
            rd1_en=False,
        ).sha(ver)
    op = dve_ops.DveOp("FRAC_SHIFT_ANT", spec, subdim=False, uops_sha=shas)
    dve_ops.OPS.append(op)
    dve_ops.CUSTOM_DVE_SPECS[op.name] = op.spec
    dve_ops._SUB_OPCODE_FOR_NAME[op.name] = opcode
    return op


def _build():
    import concourse.bacc as bacc
    import concourse.mybir as mybir
    from concourse.tile import TileContext

    frac_op = _register_frac_op()

    f32 = mybir.dt.float32
    f16 = mybir.dt.float16
    AF = mybir.ActivationFunctionType

    nc = bacc.Bacc("TRN2", target_bir_lowering=False, debug=False, num_devices=8)
    with TileContext(nc) as tc:
        kqin = nc.dram_tensor("kqin", [C, NK + NQ], f16, kind="ExternalInput")
        valtin = nc.dram_tensor("valtin", [NK + 1, C], f16, kind="ExternalInput")
        wkq = nc.dram_tensor("wkq", [C, 256], f16, kind="ExternalInput")
        # uv: cols 0:NQ = ubc (u[cin] bcast over q), NQ:NQ+128 = v2bc
        uvin = nc.dram_tensor("uvin", [C, NQ + 128], f16, kind="ExternalInput")
        # biasrow: rank-2 bias add into pkq. cols 0:864 = rhs ([ones|0;0|ones]
        # masked), 864:992 = lhsT ([bk-dup; bq-dup])
        biasrow = nc.dram_tensor("biasrow", [2, NK + NQ + 128], f16, kind="ExternalInput")
        # cpack f32: cols 0:R = qscale (+-b_r wf dup), R = phase vec (1/8,
        # 3/8 turns), R+1 = bfv, R+2:R+4 = vsum05 per ct
        cpack = nc.dram_tensor("cpack", [128, R + 4], f32, kind="ExternalInput")
        outd = nc.dram_tensor("out", [C, NQ], f16, kind="ExternalOutput")

        with (
            tc.tile_pool(name="consts", bufs=1) as consts,
            tc.tile_pool(name="inp", bufs=1) as inp,
            tc.tile_pool(name="work", bufs=1) as work,
            tc.tile_pool(name="wp", bufs=2) as wp,
            tc.tile_pool(name="fp", bufs=3) as fp,
            tc.tile_pool(name="gp", bufs=2) as gp,
            tc.tile_pool(name="psum", bufs=1, space="PSUM") as psum,
        ):
            kq_sb = [inp.tile([128, NK + NQ], f16, tag=f"kq{t}", name=f"kq{t}") for t in range(2)]
            vt_sb = [
                inp.tile([KT_SIZES[kt] + (kt == 4), C], f16, tag=f"vt{kt}", name=f"vt{kt}")
                for kt in range(5)
            ]
            wkq_sb = [consts.tile([128, 256], f16, tag=f"wkq{t}", name=f"wkq{t}") for t in range(2)]
            uv_sb = [consts.tile([128, NQ + 128], f16, tag=f"uv{t}", name=f"uv{t}") for t in range(2)]
            br_sb = consts.tile([2, NK + NQ + 128], f16, tag="br")
            cp_sb = consts.tile([128, R + 4], f32, tag="cp")
            scr = consts.tile([128, 1], f32, tag="scr")
            warm = consts.tile([128, 512], f16, tag="warm")

            # DMA issue order matters per queue (SP / ACT / Pool streams).
            nc.sync.dma_start(out=br_sb[:], in_=biasrow.ap())
            nc.scalar.dma_start(out=wkq_sb[0][:], in_=wkq.ap()[0:128, :])
            nc.gpsimd.dma_start(out=kq_sb[1][:], in_=kqin.ap()[128:256, :])
            nc.sync.dma_start(out=kq_sb[0][:], in_=kqin.ap()[0:128, :])
            nc.scalar.dma_start(out=wkq_sb[1][:], in_=wkq.ap()[128:256, :])
            nc.sync.dma_start(out=cp_sb[:], in_=cpack.ap())
            nc.sync.dma_start(out=uv_sb[0][:], in_=uvin.ap()[0:128, :])
            nc.scalar.dma_start(out=uv_sb[1][:], in_=uvin.ap()[128:256, :])
            # warmup: Silu pins the silu_and_others ACT table (sin+tanh+silu
            # in one table -> no mid-kernel ACT_TABLE_LOADs)
            nc.vector.memset(scr[:], 0.0)
            nc.scalar.activation(scr[:], scr[:], AF.Silu)
            nc.vector.memset(warm[:], 0.0)
            for kt in range(5):
                nc.gpsimd.dma_start(
                    out=vt_sb[kt][:],
                    in_=valtin.ap()[kt * 128 : kt * 128 + KT_SIZES[kt] + (kt == 4), :],
                )

            # attn4/vt4 carry a 65th row (ones / 0.5*vsum) so the value
            # matmul adds the sigmoid 0.5-offset correction for free
            attn_sb = [
                work.tile([KT_SIZES[kt] + (kt == 4), NQ], f16, tag=f"attn{kt}", name=f"attn{kt}")
                for kt in range(5)
            ]
            nc.vector.memset(attn_sb[4][64:65, :], 1.0)
            osb = [work.tile([128, NQ], f16, tag=f"osb{t}", name=f"osb{t}") for t in range(2)]

            pkq_t = psum.tile([128, NK + NQ], f32, tag="pkq")
            scores = [
                psum.tile([KT_SIZES[kt], NQ], f32, tag=f"sc{kt}", name=f"sc{kt}")
                for kt in range(5)
            ]

            # PE p-state warmup in the DMA shadow (throwaway group in pkq_t)
            for i in range(8):
                nc.tensor.matmul(
                    out=pkq_t[:, 0:512], lhsT=warm[:, 0:128], rhs=warm[:],
                    start=(i == 0), stop=(i == 7),
                )

            # k_/q_ = [W|W]^T @ (key|qry) + rank-2 bias add -> dup rows psum
            for c0, c1 in ((NK, NK + NQ), (0, 512), (512, NK)):
                col = 0 if c1 <= NK else 128
                nc.tensor.matmul(
                    out=pkq_t[:, c0:c1],
                    lhsT=br_sb[:, NK + NQ : NK + NQ + 128],
                    rhs=br_sb[:, c0:c1],
                    start=True, stop=False, skip_group_check=True,
                )
                for ct in range(2):
                    nc.tensor.matmul(
                        out=pkq_t[:, c0:c1],
                        lhsT=wkq_sb[ct][:, col : col + 128],
                        rhs=kq_sb[ct][:, c0:c1],
                        start=False, stop=(ct == 1), skip_group_check=True,
                    )

            # linear term, from the raw inputs (PE-ready at DMA land):
            # scores[k,q] += u^T key (bcast q) + v2^T qry (bcast k)
            for kt in range(5):
                ks = slice(kt * 128, kt * 128 + KT_SIZES[kt])
                for ct in range(2):
                    nc.tensor.matmul(
                        out=scores[kt][:], lhsT=kq_sb[ct][:, ks],
                        rhs=uv_sb[ct][:, 0:NQ],
                        start=(ct == 0), stop=False, skip_group_check=True,
                    )
                for ct in range(2):
                    nc.tensor.matmul(
                        out=scores[kt][:],
                        lhsT=uv_sb[ct][:, NQ : NQ + KT_SIZES[kt]],
                        rhs=kq_sb[ct][:, NK : NK + NQ],
                        start=False, stop=False, skip_group_check=True,
                    )

            # Fourier features: one FRAC_SHIFT per r over [k|q] (the pi/4
            # phase trick makes both sides share one phase vector; the sign
            # lands in qscale), one Sin ACT per r, one q-scale per r.
            wr_t = [None] * R

            def emit_wrap(r):
                wr = wp.tile([128, NK + NQ], f32, tag="wr")
                nc.vector._custom_dve(
                    frac_op, out=wr[:], in0=pkq_t[:],
                    s0=float(OM[r] / TWO_PI), s1=cp_sb[:, R : R + 1], imm2=MAGIC,
                )
                wr_t[r] = wr

            fr_t = [None] * R
            gr_t = [None] * R

            def emit_gr(r):
                gr = gp.tile([128, NQ], f16, tag="gr")
                nc.vector.tensor_scalar_mul(
                    out=gr[:], in0=fr_t[r][:, NK : NK + NQ],
                    scalar1=cp_sb[:, r : r + 1],
                )
                gr_t[r] = gr

            def emit_tail(r):
                if gr_t[r] is None:
                    emit_gr(r)
                fr, gr = fr_t[r], gr_t[r]
                for kt in range(5):
                    ks = slice(kt * 128, kt * 128 + KT_SIZES[kt])
                    nc.tensor.matmul(
                        out=scores[kt][:], lhsT=fr[:, ks], rhs=gr[:],
                        start=False, stop=(r == R - 1),
                        skip_group_check=True,
                    )

            # r=0 split q|k: the q-side features (which gate gr0 and all
            # score matmuls) chase the early-closing q region of pkq
            wr0 = wp.tile([128, NK + NQ], f32, tag="wr")
            s00 = float(OM[0] / TWO_PI)
            nc.vector._custom_dve(
                frac_op, out=wr0[:, NK : NK + NQ], in0=pkq_t[:, NK : NK + NQ],
                s0=s00, s1=cp_sb[:, R : R + 1], imm2=MAGIC,
            )
            nc.vector._custom_dve(
                frac_op, out=wr0[:, 0:NK], in0=pkq_t[:, 0:NK],
                s0=s00, s1=cp_sb[:, R : R + 1], imm2=MAGIC,
            )
            wr_t[0] = wr0
            fr0 = fp.tile([128, NK + NQ], f16, tag="fr")
            fr_t[0] = fr0
            nc.scalar.activation(
                fr0[:, NK : NK + NQ], wr0[:, NK : NK + NQ], AF.Sin, scale=TWO_PI
            )
            emit_gr(0)
            nc.scalar.activation(fr0[:, 0:NK], wr0[:, 0:NK], AF.Sin, scale=TWO_PI)
            emit_wrap(1)
            for r in range(1, R):
                fr = fp.tile([128, NK + NQ], f16, tag="fr")
                nc.scalar.activation(fr[:], wr_t[r][:], AF.Sin, scale=TWO_PI)
                fr_t[r] = fr
                if r + 1 < R:
                    emit_wrap(r + 1)
                emit_tail(r - 1)
            emit_tail(R - 1)

            # attn_t = tanh(0.5*scores + bfv); sigmoid = 0.5 + 0.5*attn_t
            for kt in range(5):
                nc.scalar.activation(
                    attn_sb[kt][: KT_SIZES[kt], :], scores[kt][:], AF.Tanh,
                    scale=0.5, bias=cp_sb[: KT_SIZES[kt], R + 1 : R + 2],
                )

            # out = (0.5*value | 0.5*vsum) @ (attn_t | ones): the 65th row of
            # the kt=4 pair adds the 0.5*vsum offset; output DMAs straight
            # from psum (reusing scores[ct]'s bank)
            for ct in range(2):
                po = scores[ct]
                for kt in range(5):
                    nc.tensor.matmul(
                        out=po[:],
                        lhsT=vt_sb[kt][:, ct * 128 : (ct + 1) * 128],
                        rhs=attn_sb[kt][:],
                        start=(kt == 0), stop=(kt == 4),
                        skip_group_check=True,
                    )
                if ct == 0:
                    nc.vector.tensor_copy(out=osb[ct][:], in_=po[:])
                else:
                    nc.scalar.activation(osb[ct][:], po[:], AF.Identity)
                (nc.sync if ct == 0 else nc.scalar).dma_start(
                    out=outd.ap()[ct * 128 : (ct + 1) * 128, :], in_=osb[ct][:]
                )
    nc.finalize()
    return nc


def _prep_in_maps(key, query, value, Wk, bk, Wq, bq, wf, bf):
    f32, f16 = np.float32, np.float16
    key = np.ascontiguousarray(key, f32).reshape(B, C, NK)
    query = np.ascontiguousarray(query, f32).reshape(B, C, HW, HW)
    value = np.ascontiguousarray(value, f32).reshape(B, C, NK)
    Wk = np.asarray(Wk, f32)
    Wq = np.asarray(Wq, f32)
    wf = np.asarray(wf, f32)
    bk = np.asarray(bk, f32)
    bq = np.asarray(bq, f32)
    bf = np.float32(bf)

    wkt2 = np.concatenate([Wk.T, Wk.T], axis=1)  # (256, 128)
    wqt2 = np.concatenate([Wq.T, Wq.T], axis=1)
    wkq = np.ascontiguousarray(np.concatenate([wkt2, wqt2], axis=1)).astype(f16)

    # linear-term vectors (biases folded into bf_eff)
    u = (A_LIN * wf) @ Wk   # (256,)
    v2 = (A_LIN * wf) @ Wq
    uv = np.zeros((C, NQ + 128), f32)
    uv[:, 0:NQ] = u[:, None]
    uv[:, NQ : NQ + 128] = v2[:, None]
    uv = np.ascontiguousarray(uv).astype(f16)

    # rank-2 bias add into the duplicated k_/q_ psum
    brow = np.zeros((2, NK + NQ + 128), f32)
    brow[0, 0:NK] = 1.0
    brow[1, NK : NK + NQ] = 1.0
    brow[0, NK + NQ :] = np.tile(bk, 2)
    brow[1, NK + NQ :] = np.tile(bq, 2)
    brow = brow.astype(f16)

    cpk = np.zeros((128, R + 4), f32)
    for r in range(R):
        cpk[:64, r] = BB[r] * wf      # + sign: sin(x+pi/4) rows
        cpk[64:, r] = -BB[r] * wf     # - sign: sin(x+3pi/4)=cos(x+pi/4) rows
    cpk[:64, R] = 0.125               # phase, in turns
    cpk[64:, R] = 0.375
    bf_eff = bf + A_LIN * float(wf @ (bk + bq))
    cpk[:, R + 1] = 0.5 * bf_eff

    key16 = key.astype(f16)
    query16 = query.astype(f16)
    common = {"wkq": wkq, "uvin": uv, "biasrow": brow}
    in_maps = []
    for i in range(8):
        b, h = i // 2, i % 2
        qs = np.ascontiguousarray(
            query16[b, :, h * 12 : (h + 1) * 12, :]
        ).reshape(C, NQ)
        vsum = 0.5 * value[b].sum(axis=1)
        valt05 = np.ascontiguousarray(
            np.concatenate([(0.5 * value[b]).T, vsum[None, :]], axis=0)
        ).astype(f16)
        m = {
            "kqin": np.ascontiguousarray(np.concatenate([key16[b], qs], axis=1)),
            "valtin": valt05,
            "cpack": cpk,
        }
        m.update(common)
        in_maps.append(m)
    return in_maps


def run(trace=False, **inputs):
    from concourse.bass_utils import run_bass_kernel_spmd

    inputs.pop("mode", None)
    inputs.pop("chunk", None)
    if "nc" not in _cache:
        _cache["nc"] = _build()
    nc = _cache["nc"]
    in_maps = _prep_in_maps(**inputs)
    res = run_bass_kernel_spmd(nc, in_maps, core_ids=list(range(8)), trace=trace)
    out = np.empty((B, C, HW, HW), np.float32)
    for i in range(8):
        b, h = i // 2, i % 2
        out[b, :, h * 12 : (h + 1) * 12, :] = (
            res.results[i]["out"].astype(np.float32).reshape(C, 12, HW)
        )
    return out, res


def kernel(**inputs):
    out, _ = run(**inputs)
    return out
